# revision 13
# baseline (speedup 1.0000x reference)
"""GQA attention (B=2, L=2048, HID=2048, 32 Q heads / 8 KV heads) on 8 TRN2 cores.

Sharding: data-parallel on batch (2) x tensor-parallel on heads (4).
Core c: batch b = c//4, TP rank r = c%4 owns q heads {8r..8r+7} (whole GQA
groups: kv heads 2r, 2r+1). bf16 TensorEngine compute, fp32 PSUM, fp32
softmax statistics. Per-core pipeline:
  1. KV proj (streamed kvT pieces, padded k-tiles skipped entirely):
     kT [128, Lk] and per-tile v [128, 130] (+ones cols for the softmax
     denominator ride-along).
  2. Q proj from SBUF-resident qT: QT[pr] = [128, L] bf16, head-pair rows.
  3. Attention per (j, pr): software-pipelined QK -> exp -> (band mul) -> PV
     with the QK of step i+1 emitted before PV of step i so the PE never
     stalls behind the scalar-engine exp. Diagonal band tiles are
     column-trimmed (leading fully-masked q columns skipped in exp/PV/QK).
     Normalized attn halves are DMA'd straight to the AllGather input
     ag_in[j] in DRAM (no local output projection of own heads).
  4. Output side in AllGather form: ag_in[j] [512,512] -> AllGather over the
     4-core TP group -> ag_out[j] [2048, 512] (every core gets all heads'
     attention for the chunk). The o-projection is FULLY DEFERRED: after the
     last attention chunk the PE runs oproj for all 4 chunks back-to-back
     (local f-slab [512 f] x [512 q] per chunk, contraction over all 2048 d
     from SBUF-resident gathered atf tiles), hiding the tail AllGather under
     deferred compute. Chunk 3's AllGather is split by head-pair halves so
     the first half is triggered mid-attention.
Host assembles [2, 2048, 2048] f32 from per-core [4][512, 512] bf16 slabs.

Mask handling is input-driven: blocks are classified all-masked (skipped),
all-zero (no mask op), or band (exp(mask) multiplied into exp(scores));
band tiles additionally get a leading-dead-column trim q0.
"""

import numpy as np
import ml_dtypes
import concourse.bass as bass
import concourse.mybir as mybir
import concourse.tile as tile
from concourse import bacc
from concourse.bass_utils import run_bass_kernel_spmd

F32 = mybir.dt.float32
BF16 = mybir.dt.bfloat16
AF = mybir.ActivationFunctionType
NPBF16 = ml_dtypes.bfloat16

B, L, HID = 2, 2048, 2048
NH, D, NKV = 32, 64, 8
SCALE = 0.125
N_CORES = 8
TPR = 4          # TP ranks per batch group
NPAIR = 4        # head pairs per core
LQC = 512        # q chunk (PSUM-bank sized)
NJ = L // LQC    # 4
KT = 128         # k-position tile
NI = L // KT     # 16
NKC = HID // 128  # 16 contraction chunks
NEG_THRESH = -1.0e8

_graph_cache = {}
last_results = None  # BassKernelResults of the most recent run (for test harness)


def _classify_blocks(eff_masks):
    """eff_masks: list of B arrays [L, L] (q, k). Returns (live, band_list,
    trim) where live[j] lists live k-tiles for q-chunk j, band_list orders
    blocks needing explicit mask values, and trim[(j, i)] is the count of
    leading q-columns of the transposed block that are fully masked."""
    live = {}
    band_list = []
    trim = {}
    for j in range(NJ):
        lv = []
        for i in range(NI):
            subs = [m[j * LQC:(j + 1) * LQC, i * KT:(i + 1) * KT] for m in eff_masks]
            if all((s <= NEG_THRESH).all() for s in subs):
                continue  # fully masked in every batch
            lv.append(i)
            # leading q-columns (rows of the [LQC, KT] block) dead in all b
            dead_q = np.logical_and.reduce(
                [(s <= NEG_THRESH).all(axis=1) for s in subs])
            q0 = 0
            while q0 < LQC and dead_q[q0]:
                q0 += 1
            q0 &= ~127  # keep alignment coarse; only full-128 steps trimmed
            trim[(j, i)] = q0
            if not all((s == 0.0).all() for s in subs):
                band_list.append((j, i))
        live[j] = lv
    return live, band_list, trim


def _build_graph(live_key, band_key, trim_key, live_k):
    key = (live_key, band_key, trim_key, tuple(live_k))
    if key in _graph_cache:
        return _graph_cache[key]

    live = {j: list(lv) for j, lv in live_key}
    band_list = list(band_key)
    trim = dict(trim_key)
    band_idx = {ji: n for n, ji in enumerate(band_list)}
    nb = max(1, len(band_list))
    nt = len(live_k)              # live k tiles
    Lk = nt * KT
    pos_of = {i: t for t, i in enumerate(live_k)}
    NSK = (Lk + 511) // 512       # kT column sub-blocks

    nc = bacc.Bacc("TRN2", target_bir_lowering=False, debug=False,
                   num_devices=N_CORES)

    # host-prelayouted inputs: [128, ...] sbuf-shaped flat rows
    qT = nc.dram_tensor("qT", [128, NKC * L], BF16, kind="ExternalInput")
    kvT = nc.dram_tensor("kvT", [128, NKC * Lk], BF16, kind="ExternalInput")
    wq = nc.dram_tensor("wq", [128, NKC * 512], BF16, kind="ExternalInput")
    # wkv: chunk k -> cols [256k:256k+128] = wk, [256k+128:256k+256] = wv
    wkv = nc.dram_tensor("wkv", [128, NKC * 256], BF16, kind="ExternalInput")
    # wo: gathered-row order (rank, pair, half, d) x own 512-col f-slab
    wo = nc.dram_tensor("wo", [128, NKC * 512], BF16, kind="ExternalInput")
    # consts_bf: cols 0:128 ones; row 0 cols 128:640 = bv4
    consts_bf = nc.dram_tensor("consts_bf", [128, 640], BF16,
                               kind="ExternalInput")
    # consts_f32: cols 0:4 bq, col 4 bk, cols 5:9 bo f-slab
    consts_f32 = nc.dram_tensor("consts_f32", [128, 21], F32,
                                kind="ExternalInput")
    band = nc.dram_tensor("band", [128, nb * LQC], BF16, kind="ExternalInput")

    ag_warm_in = nc.dram_tensor("ag_warm_in", [32, 16], BF16)
    ag_warm_out = nc.dram_tensor("ag_warm_out", [128, 16], BF16)
    ag_in = [nc.dram_tensor(f"ag_in{j}", [512, LQC], BF16)
             for j in range(NJ - 1)]
    ag_out = [nc.dram_tensor(f"ag_out{j}", [4 * 512, LQC], BF16)
              for j in range(NJ - 1)]
    # chunk 3 AllGathers split by pair halves so the first half launches
    # mid-attention, shrinking the tail exposure
    ag_in3 = [nc.dram_tensor(f"ag_in3_{h}", [256, LQC], BF16)
              for h in range(2)]
    ag_out3 = [nc.dram_tensor(f"ag_out3_{h}", [4 * 256, LQC], BF16)
               for h in range(2)]
    out_ext = [nc.dram_tensor(f"out{j}", [512, LQC], BF16,
                              kind="ExternalOutput") for j in range(NJ)]
    groups = [[0, 1, 2, 3], [4, 5, 6, 7]]

    with tile.TileContext(nc) as tc:
        with tc.tile_pool(name="persist", bufs=1) as persist:
            # DMA plan (issue cost ~2.5us/DMA per queue -> few, big, ordered):
            #  sync:   consts_bf, kv pieces, consts_f32, wq, qt pieces, band, wo
            #  scalar: wkv, ats->ag_in during attn, atf gathers
            #  gpsimd: collective warmup + AG triggers
            cbf_sb = persist.tile([128, 640], BF16, tag="cbf")
            cf_sb = persist.tile([128, 21], F32, tag="cf")
            wq_sb = persist.tile([128, NKC * 512], BF16, tag="wq")
            wo_sb = persist.tile([128, NKC * 512], BF16, tag="wo")
            band_sb = persist.tile([128, nb * LQC], BF16, tag="band")
            kT_sb = persist.tile([128, Lk], BF16, tag="kT")
            v_sb = [persist.tile([128, 130], BF16, tag=f"v{t}", name=f"v{t}")
                    for t in range(nt)]
            QT_sb = [[persist.tile([128, 1024], BF16, tag=f"qt{m}{jp}",
                                   name=f"qt{m}{jp}") for jp in range(2)]
                     for m in range(NPAIR)]
            ones_sb = cbf_sb[:, 0:128]
            bv4_sb = cbf_sb[0:1, 128:640]
            bq_sb = cf_sb[:, 0:4]
            bk_sb = cf_sb[:, 4:5]
            bo_sb = cf_sb[:, 5:9]

            with tc.tile_pool(name="qtp_scope", bufs=1) as qtsc:
                wkv_sb = qtsc.tile([128, NKC * 256], BF16, tag="wkv")
                # qt staging read by the Q jp1 sub-eras inside attention
                # scope 1, so it lives at qtp_scope level.
                qtp = [[qtsc.tile([128, 8 * 1024], BF16, tag=f"qtp{jp}{h}",
                                  name=f"qtp{jp}{h}") for h in range(2)]
                       for jp in range(2)]

                nc.sync.dma_start(cbf_sb[:], consts_bf[:])
                nc.scalar.dma_start(wkv_sb[:], wkv[:])
                for t in range(nt):
                    nc.vector.tensor_copy(v_sb[t][:, 64:65], ones_sb[:, 0:1])
                    nc.vector.tensor_copy(v_sb[t][:, 129:130], ones_sb[:, 0:1])

                # ---- KV projection (kvch era-scoped; queue-ordered DMAs)
                with (
                    tc.tile_pool(name="kv_stream", bufs=1) as kvs,
                    tc.tile_pool(name="kv_psum", bufs=1, space="PSUM") as kvp,
                ):
                    # ALL critical input transfers go on the sync ring in
                    # exact consumption order -- SDMA round-robins *rings* at
                    # packet granularity, so priority only exists within one
                    # ring.
                    kvpc = [kvs.tile([128, 2 * Lk], BF16, tag=f"kvpc{p}",
                                     name=f"kvpc{p}") for p in range(8)]
                    for p in range(8):
                        nc.sync.dma_start(kvpc[p][:],
                                          kvT[:, 2 * p * Lk:2 * (p + 1) * Lk])
                    nc.sync.dma_start(cf_sb[:], consts_f32[:])
                    nc.sync.dma_start(wq_sb[:], wq[:])
                    for jp in range(2):
                        for h in range(2):
                            nc.sync.dma_start(
                                qtp[jp][h][:],
                                qT[:, (2 * jp + h) * 8192:
                                      (2 * jp + h + 1) * 8192])
                    # band/wo after the hot inputs on the same ring
                    nc.sync.dma_start(band_sb[:], band[:])
                    nc.sync.dma_start(wo_sb[:], wo[:])
                    nc.gpsimd.dma_start(ag_warm_in[:], consts_bf[0:32, 0:16])
                    nc.gpsimd.collective_compute(
                        "AllGather", mybir.AluOpType.bypass,
                        replica_groups=groups,
                        ins=[ag_warm_in[:]], outs=[ag_warm_out[:]])
                    wids = [min(512, Lk - 512 * s) for s in range(NSK)]
                    psk = [kvp.tile([128, wids[s]], F32, tag=f"psk{s}",
                                    name=f"psk{s}") for s in range(NSK)]
                    psv = [kvp.tile([128, wids[s]], F32, tag=f"psv{s}",
                                    name=f"psv{s}") for s in range(NSK)]
                    for s in range(NSK):
                        nc.tensor.matmul(psv[s][:], ones_sb[0:1, :],
                                         bv4_sb[:, 0:wids[s]], start=True,
                                         stop=False, skip_group_check=True)
                    for k in range(NKC):
                        kv_ch = kvpc[k // 2]
                        off = (k % 2) * Lk
                        for s in range(NSK):
                            nc.tensor.matmul(
                                psk[s][:], wkv_sb[:, 256 * k:256 * k + 128],
                                kv_ch[:, off + 512 * s:
                                      off + 512 * s + wids[s]],
                                start=(k == 0), stop=(k == NKC - 1))
                        for t in range(nt):
                            s, col = t // 4, t % 4
                            nc.tensor.matmul(
                                psv[s][:, 128 * col:128 * (col + 1)],
                                kv_ch[:, off + 128 * t:off + 128 * (t + 1)],
                                wkv_sb[:, 256 * k + 128:256 * (k + 1)],
                                start=False, stop=(k == NKC - 1),
                                skip_group_check=True)
                    for s in range(NSK):
                        nc.scalar.activation(
                            kT_sb[:, 512 * s:512 * s + wids[s]],
                            psk[s][:], AF.Identity, bias=bk_sb[:])
                    for t in range(nt):
                        s, col = t // 4, t % 4
                        nc.scalar.copy(v_sb[t][:, 0:64],
                                       psv[s][:, 128 * col:128 * col + 64])
                        nc.vector.tensor_copy(
                            v_sb[t][:, 65:129],
                            psv[s][:, 128 * col + 64:128 * (col + 1)])

                # ---- Q projection from resident qT (jp-major layout).
                # jp=1 is interleaved into attention chunk 0 (scope 1 below).
                with tc.tile_pool(name="q_psum", bufs=1, space="PSUM") as qp:
                    for jp in range(1):
                        psq = [qp.tile([128, 512], F32, tag=f"psq{n}",
                                       name=f"psq{n}") for n in range(8)]
                        for k in range(NKC):
                            qch = qtp[jp][k // 8]
                            off = (k % 8) * 1024
                            for m in range(NPAIR):
                                for jj in range(2):
                                    nc.tensor.matmul(
                                        psq[4 * jj + m][:],
                                        wq_sb[:, 512 * k + 128 * m:
                                                 512 * k + 128 * (m + 1)],
                                        qch[:, off + 512 * jj:
                                            off + 512 * (jj + 1)],
                                        start=(k == 0), stop=(k == NKC - 1))
                        for jj in range(2):
                            for m in range(NPAIR):
                                nc.scalar.activation(
                                    QT_sb[m][jp][:, 512 * jj:512 * (jj + 1)],
                                    psq[4 * jj + m][:], AF.Identity,
                                    bias=bq_sb[:, m:m + 1])

                # ---- Attention scope 1: chunk 0 with Q jp=1 sub-eras.
                P = {}
                # two-stage deferred normalize for scope 2: stage A broadcasts
                # the reciprocal row via a PE ones-matmul into PSUM one block
                # after its recips; stage B (muls + at->DRAM exports) runs one
                # block after that, so no PE-feeding queue ever waits on a
                # laggy cross-engine producer.
                norm_bc = []   # (ra, rb, ua, ub, dst, row0)
                norm_mul = []  # (ua, ub, rba, rbb, dst, row0)

                def flush_norm():
                    if norm_mul:
                        ua, ub, rba, rbb, dst, row0 = norm_mul.pop(0)
                        at_a = P["at_pool"].tile([64, 512], BF16, tag="at_a")
                        at_b = P["at_pool"].tile([64, 512], BF16, tag="at_b")
                        nc.vector.tensor_mul(at_a[:], ua[0:64, :], rba[:])
                        nc.vector.tensor_mul(at_b[:], ub[0:64, :], rbb[:])
                        nc.gpsimd.dma_start(dst[row0:row0 + 64, :], at_a[:])
                        nc.gpsimd.dma_start(dst[row0 + 64:row0 + 128, :],
                                            at_b[:])
                    if norm_bc:
                        ra, rb, ua, ub, dst, row0 = norm_bc.pop(0)
                        rba = P["bc_psum"].tile([64, 512], F32, tag="rba")
                        rbb = P["bc_psum"].tile([64, 512], F32, tag="rbb")
                        nc.tensor.matmul(rba[:], ones_sb[0:1, 0:64], ra[:],
                                         start=True, stop=True,
                                         skip_group_check=True)
                        nc.tensor.matmul(rbb[:], ones_sb[0:1, 0:64], rb[:],
                                         start=True, stop=True,
                                         skip_group_check=True)
                        norm_mul.append((ua, ub, rba, rbb, dst, row0))

                def drain_norm():
                    while norm_mul or norm_bc:
                        flush_norm()

                def attn_block(j, pr, dst, dst_row0, filler=None):
                    """Compute attention for (chunk j, pair pr); stage the two
                    normalized 64-row halves for export to DRAM tensor dst at
                    rows dst_row0 / dst_row0+64 (finished by the next
                    flush_norm). filler(n) emits PE work after tile n's QK so
                    the PE stays busy while the scalar engine runs exp."""
                    lv = live[j]
                    nlast = len(lv) - 1
                    pva = P["pv_psum"].tile([65, 512], F32, tag="pva")
                    pvb = P["pv_psum"].tile([65, 512], F32, tag="pvb")
                    pts = []
                    for n, i in enumerate(lv):
                        t = pos_of[i]
                        q0 = trim.get((j, i), 0)
                        qt_t = QT_sb[pr][j // 2]
                        qoff = 512 * (j % 2)
                        ps = P["qk_psum"].tile([128, 1024], F32, tag="qk")
                        nc.tensor.matmul(
                            ps[:, q0:512],
                            kT_sb[0:64, 128 * t:128 * (t + 1)],
                            qt_t[0:64, qoff + q0:qoff + 512],
                            start=True, stop=True, skip_group_check=True)
                        nc.tensor.matmul(
                            ps[:, 512 + q0:1024],
                            kT_sb[64:128, 128 * t:128 * (t + 1)],
                            qt_t[64:128, qoff + q0:qoff + 512],
                            start=True, stop=True, skip_group_check=True)
                        pt = P["pt_pool"].tile([128, 1024], BF16, tag="pt")
                        if q0 == 0:
                            nc.scalar.activation(pt[:], ps[:], AF.Exp)
                        else:
                            nc.scalar.activation(pt[:, q0:512],
                                                 ps[:, q0:512], AF.Exp)
                            nc.scalar.activation(pt[:, 512 + q0:1024],
                                                 ps[:, 512 + q0:1024], AF.Exp)
                        if (j, i) in band_idx:
                            bcol = band_idx[(j, i)] * LQC
                            nc.vector.tensor_mul(
                                pt[:, q0:512], pt[:, q0:512],
                                band_sb[:, bcol + q0:bcol + 512])
                            nc.vector.tensor_mul(
                                pt[:, 512 + q0:1024], pt[:, 512 + q0:1024],
                                band_sb[:, bcol + q0:bcol + 512])
                        pts.append((pt, q0))
                        if filler is not None:
                            filler(n)
                        if n == 2:
                            # previous block's normalize finish lands here,
                            # one block after its reciprocal was produced
                            flush_norm()
                        # software pipeline: PV lags QK/exp by one step
                        if n > 0:
                            ptp, q0p = pts[n - 1]
                            tp = pos_of[lv[n - 1]]
                            nc.tensor.matmul(
                                pva[:, q0p:512], v_sb[tp][:, 0:65],
                                ptp[:, q0p:512], start=(n - 1 == 0),
                                stop=False, skip_group_check=True)
                            nc.tensor.matmul(
                                pvb[:, q0p:512], v_sb[tp][:, 65:130],
                                ptp[:, 512 + q0p:1024], start=(n - 1 == 0),
                                stop=False, skip_group_check=True)
                    ptp, q0p = pts[nlast]
                    tp = pos_of[lv[nlast]]
                    nc.tensor.matmul(
                        pva[:, q0p:512], v_sb[tp][:, 0:65], ptp[:, q0p:512],
                        start=(nlast == 0), stop=True, skip_group_check=True)
                    nc.tensor.matmul(
                        pvb[:, q0p:512], v_sb[tp][:, 65:130],
                        ptp[:, 512 + q0p:1024], start=(nlast == 0), stop=True,
                        skip_group_check=True)
                    # unload + normalize (vector only; scalar stays pure exp)
                    ua = P["ua_pool"].tile([65, 512], F32, tag="ua")
                    ub = P["ua_pool"].tile([65, 512], F32, tag="ub")
                    nc.vector.tensor_copy(ua[:], pva[:])
                    nc.vector.tensor_copy(ub[:], pvb[:])
                    # den rows to partition 0 (DVE cannot partition-shift
                    # SBUF->SBUF; PSUM row reads to p0 are fine)
                    rsa = P["rc_pool"].tile([1, 512], F32, tag="rsa")
                    rsb = P["rc_pool"].tile([1, 512], F32, tag="rsb")
                    nc.vector.tensor_copy(rsa[:], pva[64:65, :])
                    nc.vector.tensor_copy(rsb[:], pvb[64:65, :])
                    ra = P["rc_pool"].tile([1, 512], F32, tag="ra")
                    rb = P["rc_pool"].tile([1, 512], F32, tag="rb")
                    nc.vector.reciprocal_approx_fast(out=ra[:], in_=rsa[:])
                    nc.vector.reciprocal_approx_fast(out=rb[:], in_=rsb[:])
                    if P.get("defer"):
                        # bf16 copies feed the PE ones-matmul broadcast
                        rah = P["rc_pool"].tile([1, 512], BF16, tag="rah")
                        rbh = P["rc_pool"].tile([1, 512], BF16, tag="rbh")
                        nc.vector.tensor_copy(rah[:], ra[:])
                        nc.vector.tensor_copy(rbh[:], rb[:])
                        norm_bc.append((rah, rbh, ua, ub, dst, dst_row0))
                    else:
                        # chunk 0: no collective in flight, gpsimd broadcast
                        # + immediate finish is safe and cheapest
                        rba = P["rb_pool"].tile([64, 512], F32, tag="rba")
                        rbb = P["rb_pool"].tile([64, 512], F32, tag="rbb")
                        nc.gpsimd.partition_broadcast(rba[:], ra[:])
                        nc.gpsimd.partition_broadcast(rbb[:], rb[:])
                        at_a = P["at_pool"].tile([64, 512], BF16, tag="at_a")
                        at_b = P["at_pool"].tile([64, 512], BF16, tag="at_b")
                        nc.vector.tensor_mul(at_a[:], ua[0:64, :], rba[:])
                        nc.vector.tensor_mul(at_b[:], ub[0:64, :], rbb[:])
                        nc.gpsimd.dma_start(
                            dst[dst_row0:dst_row0 + 64, :], at_a[:])
                        nc.gpsimd.dma_start(
                            dst[dst_row0 + 64:dst_row0 + 128, :], at_b[:])

                def q_sub_piece(s, g, psq):
                    """Quarter g (k chunks 4g..4g+4) of Q-proj sub-era s;
                    unloads fire with the last quarter."""
                    jj, mp = s // 2, s % 2
                    for k in range(4 * g, 4 * g + 4):
                        qch = qtp[1][k // 8]
                        off = (k % 8) * 1024 + 512 * jj
                        for mi in range(2):
                            m = 2 * mp + mi
                            nc.tensor.matmul(
                                psq[:, 512 * mi:512 * (mi + 1)],
                                wq_sb[:, 512 * k + 128 * m:
                                         512 * k + 128 * (m + 1)],
                                qch[:, off:off + 512],
                                start=(k == 0), stop=(k == NKC - 1))
                    if g == 3:
                        for mi in range(2):
                            m = 2 * mp + mi
                            nc.scalar.activation(
                                QT_sb[m][1][:, 512 * jj:512 * (jj + 1)],
                                psq[:, 512 * mi:512 * (mi + 1)], AF.Identity,
                                bias=bq_sb[:, m:m + 1])

                with (
                    tc.tile_pool(name="pt1", bufs=3) as _pt1,
                    tc.tile_pool(name="ua1", bufs=2) as _ua1,
                    tc.tile_pool(name="rc1", bufs=2) as _rc1,
                    tc.tile_pool(name="rb1", bufs=2) as _rb1,
                    tc.tile_pool(name="at1", bufs=4) as _at1,
                    tc.tile_pool(name="qk1", bufs=2, space="PSUM") as _qk1,
                    tc.tile_pool(name="pv1", bufs=1, space="PSUM") as _pv1,
                    tc.tile_pool(name="qsub", bufs=1, space="PSUM") as _qs,
                ):
                    P.update(pt_pool=_pt1, ua_pool=_ua1, rc_pool=_rc1,
                             rb_pool=_rb1, at_pool=_at1,
                             qk_psum=_qk1, pv_psum=_pv1, defer=False)
                    for pr in range(NPAIR):
                        psq = _qs.tile([128, 1024], F32, tag="qsub")
                        attn_block(0, pr, ag_in[0], 128 * pr,
                                   filler=lambda g, s=pr, t=psq:
                                       q_sub_piece(s, g, t))
                    nc.gpsimd.collective_compute(
                        "AllGather", mybir.AluOpType.bypass,
                        replica_groups=groups,
                        ins=[ag_in[0][:]], outs=[ag_out[0][:]])

            # ---- Scope 2: chunks 1..3 + AGs, then deferred oproj tail.
            # atf holds all gathered chunks (read only in the tail).
            with (
                tc.tile_pool(name="atf", bufs=3) as _atf,
                tc.tile_pool(name="atf3", bufs=2) as _atf3,
            ):
                def gather_to_sbuf(pool, src_dram, a, tag):
                    # gpsimd: queued right behind the AllGather that produces
                    # src_dram, so it issues the moment the AG completes
                    t = pool.tile([128, a * 512], BF16, tag=tag)
                    dst_ap = t[:].rearrange("p (a c) -> p a c", a=a)
                    src_ap = src_dram[:].rearrange("(a p) c -> p a c", p=128)
                    nc.gpsimd.dma_start(dst_ap, src_ap)
                    return t

                atf = [None] * (NJ - 1)
                atf3 = [None] * 2

                with (
                    tc.tile_pool(name="pt2", bufs=3) as _pt2,
                    tc.tile_pool(name="ua2", bufs=3) as _ua2,
                    tc.tile_pool(name="rc2", bufs=2) as _rc2,
                    tc.tile_pool(name="at2", bufs=6) as _at2,
                    tc.tile_pool(name="qk2", bufs=2, space="PSUM") as _qk2,
                    tc.tile_pool(name="pv2", bufs=1, space="PSUM") as _pv2,
                    tc.tile_pool(name="bc2", bufs=1, space="PSUM") as _bc2,
                ):
                    P.update(pt_pool=_pt2, ua_pool=_ua2, rc_pool=_rc2,
                             at_pool=_at2, bc_psum=_bc2,
                             qk_psum=_qk2, pv_psum=_pv2, defer=True)
                    atf[0] = gather_to_sbuf(_atf, ag_out[0], 16, "atf")

                    for j in range(1, NJ - 1):
                        for pr in range(NPAIR):
                            attn_block(j, pr, ag_in[j], 128 * pr)
                        drain_norm()
                        nc.gpsimd.collective_compute(
                            "AllGather", mybir.AluOpType.bypass,
                            replica_groups=groups,
                            ins=[ag_in[j][:]], outs=[ag_out[j][:]])
                        atf[j] = gather_to_sbuf(_atf, ag_out[j], 16, "atf")
                    for h in range(2):
                        for pi in range(2):
                            attn_block(NJ - 1, 2 * h + pi, ag_in3[h],
                                       128 * pi)
                        drain_norm()
                        nc.gpsimd.collective_compute(
                            "AllGather", mybir.AluOpType.bypass,
                            replica_groups=groups,
                            ins=[ag_in3[h][:]], outs=[ag_out3[h][:]])
                        atf3[h] = gather_to_sbuf(_atf3, ag_out3[h], 8,
                                                 "atf3")

                # ---- Deferred o-projection tail: local f-slab per chunk
                # (own PSUM scope; the attention pools above are closed).
                with (
                    tc.tile_pool(name="osb2", bufs=2) as _osb2,
                    tc.tile_pool(name="o_ps", bufs=2, space="PSUM") as _ops,
                ):
                    # chunk 3's atf3 row-block m of half h holds (rank m//2,
                    # pair 2h + m%2); in full-chunk order that block sits at
                    # dc = 4*(m//2) + 2h + m%2.
                    for j in range(NJ):
                        osb = _osb2.tile([128, 4 * 512], BF16, tag="osb")
                        for fb in range(4):
                            pso = _ops.tile([128, 512], F32, tag="pso")
                            for dc in range(16):
                                if j < NJ - 1:
                                    mv = atf[j][:, 512 * dc:512 * (dc + 1)]
                                    wcol = 512 * dc + 128 * fb
                                else:
                                    h, m = dc // 8, dc % 8
                                    mv = atf3[h][:, 512 * m:512 * (m + 1)]
                                    wcol = 512 * (4 * (m // 2) + 2 * h
                                                  + m % 2) + 128 * fb
                                nc.tensor.matmul(
                                    pso[:], wo_sb[:, wcol:wcol + 128], mv,
                                    start=(dc == 0), stop=(dc == 15),
                                    skip_group_check=True)
                            sl = osb[:, 512 * fb:512 * (fb + 1)]
                            if fb % 2 == 0:
                                nc.scalar.activation(sl, pso[:], AF.Identity,
                                                     bias=bo_sb[:, fb:fb + 1])
                            else:
                                nc.vector.tensor_scalar_add(
                                    sl, pso[:], bo_sb[:, fb:fb + 1])
                        dst_ap = out_ext[j][:].rearrange("(a p) c -> p a c",
                                                         p=128)
                        src_ap = osb[:].rearrange("p (a c) -> p a c", a=4)
                        nc.sync.dma_start(dst_ap, src_ap)

    nc.compile()
    _graph_cache[key] = nc
    return nc


def _prelayout(a, width):
    """[NKC*128, width] row-major -> [128, NKC*width] sbuf layout."""
    return np.ascontiguousarray(
        a.reshape(NKC, 128, width).transpose(1, 0, 2).reshape(128, NKC * width))


def kernel(query, kv, Wq, bq, Wkv, bkv, Wo, bo, attn_mask, key_padding_mask):
    global last_results
    query = np.asarray(query, np.float32)
    kv = np.asarray(kv, np.float32)
    Wq = np.asarray(Wq, np.float32)
    bq = np.asarray(bq, np.float32)
    Wkv = np.asarray(Wkv, np.float32)
    bkv = np.asarray(bkv, np.float32)
    Wo = np.asarray(Wo, np.float32)
    bo = np.asarray(bo, np.float32)
    attn_mask = np.asarray(attn_mask, np.float32)
    kpm = np.asarray(key_padding_mask)

    eff = [attn_mask + np.where(kpm[b], np.float32(-1e9),
                                np.float32(0.0))[None, :]
           for b in range(B)]
    live, band_list, trim = _classify_blocks(eff)
    live_k = sorted({i for lv in live.values() for i in lv})
    live_key = tuple((j, tuple(lv)) for j, lv in sorted(live.items()))
    band_key = tuple(band_list)
    trim_key = tuple(sorted(trim.items()))

    nc = _build_graph(live_key, band_key, trim_key, live_k)

    nt = len(live_k)
    Lk = nt * KT

    # Host-side shard prep. qT jp-major: col = jp*16*1024 + k*1024 + c
    qTh = [np.ascontiguousarray(
        query[b].T.astype(NPBF16).reshape(NKC, 128, 2, 1024)
        .transpose(2, 1, 0, 3).reshape(2, 128, NKC * 1024)
        .transpose(1, 0, 2).reshape(128, NKC * L)) for b in range(B)]
    kvTsel = [np.ascontiguousarray(
        kv[b].T.astype(NPBF16)
        .reshape(HID, NI, KT)[:, live_k, :].reshape(HID, Lk)) for b in range(B)]
    kvTh = [_prelayout(k_, Lk) for k_ in kvTsel]
    nb = max(1, len(band_list))
    bandh = []
    with np.errstate(over="ignore", under="ignore"):
        for b in range(B):
            if band_list:
                bandh.append(np.ascontiguousarray(np.concatenate(
                    [np.exp(eff[b][j * LQC:(j + 1) * LQC,
                                   i * KT:(i + 1) * KT].T)
                     for (j, i) in band_list], axis=1).astype(NPBF16)))
            else:
                bandh.append(np.zeros((KT, nb * LQC), NPBF16))
    ones_h = np.ones((128, 128), NPBF16)

    Wq_h = Wq.reshape(HID, NH, D)
    bq_h = bq.reshape(NH, D)

    # gathered-row permutation: global row g = 512*rank + 128*pair + 64*e + d
    g = np.arange(HID)
    head_of_g = 8 * (g // 512) + (g % 512) // 128 + 4 * ((g % 128) // 64)
    row_of_g = head_of_g * D + (g % 64)

    in_maps = []
    for c in range(N_CORES):
        b, r = c // TPR, c % TPR
        heads_q = [8 * r + pr + 4 * e for pr in range(NPAIR) for e in range(2)]
        wq_c = _prelayout(
            (Wq_h[:, heads_q, :].reshape(HID, 512) * SCALE).astype(NPBF16),
            512)
        bq_c = (bq_h[heads_q].reshape(512) * SCALE).reshape(4, 128).T
        wk_c = Wkv[:, 128 * r:128 * (r + 1)].astype(NPBF16)
        bk_c = bkv[128 * r:128 * (r + 1)]
        wv_c = Wkv[:, 512 + 128 * r:512 + 128 * (r + 1)].astype(NPBF16)
        # wkv interleave: chunk k -> [wk_k | wv_k]
        wkv_c = np.ascontiguousarray(np.concatenate(
            [np.concatenate([wk_c.reshape(NKC, 128, 128)[k],
                             wv_c.reshape(NKC, 128, 128)[k]], axis=1)
             for k in range(NKC)], axis=1))  # [128, NKC*256]
        bv_c = bkv[512 + 128 * r:512 + 128 * (r + 1)]
        # wo: gathered-row order x own f-slab columns
        wo_c = _prelayout(
            Wo[row_of_g, 512 * r:512 * (r + 1)].astype(NPBF16), 512)
        cbf = np.zeros((128, 640), NPBF16)
        cbf[:, 0:128] = ones_h
        cbf[0, 128:640] = np.tile(bv_c, 4).astype(NPBF16)
        cf = np.zeros((128, 21), np.float32)
        cf[:, 0:4] = bq_c
        cf[:, 4] = bk_c
        cf[:, 5:9] = bo[512 * r:512 * (r + 1)].reshape(4, 128).T
        in_maps.append({
            "qT": qTh[b], "kvT": kvTh[b],
            "wq": wq_c, "wkv": wkv_c, "wo": wo_c,
            "consts_bf": np.ascontiguousarray(cbf),
            "consts_f32": np.ascontiguousarray(cf),
            "band": bandh[b],
        })

    last_results = run_bass_kernel_spmd(nc, in_maps,
                                        core_ids=list(range(N_CORES)))

    out = np.empty((B, L, HID), np.float32)
    for c in range(N_CORES):
        b, r = c // TPR, c % TPR
        for j in range(NJ):
            out[b, 512 * j:512 * (j + 1), 512 * r:512 * (r + 1)] = \
                last_results.results[c][f"out{j}"].T.astype(np.float32)
    return out


# revision 14
# speedup vs baseline: 1.0843x; 1.0843x over previous
"""GQA attention (B=2, L=2048, HID=2048, 32 Q heads / 8 KV heads) on 8 TRN2 cores.

Sharding: data-parallel on batch (2) x tensor-parallel on heads (4).
Core c: batch b = c//4, TP rank r = c%4 owns q heads {8r..8r+7} (whole GQA
groups: kv heads 2r, 2r+1). bf16 TensorEngine compute, fp32 PSUM, fp32
softmax statistics. Per-core pipeline:
  1. KV proj (streamed kvT pieces, padded k-tiles skipped entirely):
     kT [128, Lk] and per-tile v [128, 130] (+ones cols for the softmax
     denominator ride-along).
  2. Q proj from SBUF-resident qT: QT[pr] = [128, L] bf16, head-pair rows.
  3. Attention per (j, pr): software-pipelined QK -> exp -> (band mul) -> PV
     with the QK of step i+1 emitted before PV of step i so the PE never
     stalls behind the scalar-engine exp. Diagonal band tiles are
     column-trimmed (leading fully-masked q columns skipped in exp/PV/QK).
     Normalized attn halves are DMA'd straight to the AllGather input
     ag_in[j] in DRAM (no local output projection of own heads).
  4. Output side in AllGather form: ag_in[j] [512,512] -> AllGather over the
     4-core TP group -> ag_out[j] [2048, 512] (every core gets all heads'
     attention for the chunk). The o-projection is FULLY DEFERRED: after the
     last attention chunk the PE runs oproj for all 4 chunks back-to-back
     (local f-slab [512 f] x [512 q] per chunk, contraction over all 2048 d
     from SBUF-resident gathered atf tiles), hiding the tail AllGather under
     deferred compute. Chunk 3's AllGather is split by head-pair halves so
     the first half is triggered mid-attention.
Host assembles [2, 2048, 2048] f32 from per-core [4][512, 512] bf16 slabs.

Mask handling is input-driven: blocks are classified all-masked (skipped),
all-zero (no mask op), or band (exp(mask) multiplied into exp(scores));
band tiles additionally get a leading-dead-column trim q0.
"""

import numpy as np
import ml_dtypes
import concourse.bass as bass
import concourse.mybir as mybir
import concourse.tile as tile
from concourse import bacc
from concourse.bass_utils import run_bass_kernel_spmd

F32 = mybir.dt.float32
BF16 = mybir.dt.bfloat16
AF = mybir.ActivationFunctionType
NPBF16 = ml_dtypes.bfloat16

B, L, HID = 2, 2048, 2048
NH, D, NKV = 32, 64, 8
SCALE = 0.125
N_CORES = 8
TPR = 4          # TP ranks per batch group
NPAIR = 4        # head pairs per core
LQC = 512        # q chunk (PSUM-bank sized)
NJ = L // LQC    # 4
KT = 128         # k-position tile
NI = L // KT     # 16
NKC = HID // 128  # 16 contraction chunks
NEG_THRESH = -1.0e8

_graph_cache = {}
last_results = None  # BassKernelResults of the most recent run (for test harness)


def _classify_blocks(eff_masks):
    """eff_masks: list of B arrays [L, L] (q, k). Returns (live, band_list,
    trim) where live[j] lists live k-tiles for q-chunk j, band_list orders
    blocks needing explicit mask values, and trim[(j, i)] is the count of
    leading q-columns of the transposed block that are fully masked."""
    live = {}
    band_list = []
    trim = {}
    for j in range(NJ):
        lv = []
        for i in range(NI):
            subs = [m[j * LQC:(j + 1) * LQC, i * KT:(i + 1) * KT] for m in eff_masks]
            if all((s <= NEG_THRESH).all() for s in subs):
                continue  # fully masked in every batch
            lv.append(i)
            # leading q-columns (rows of the [LQC, KT] block) dead in all b
            dead_q = np.logical_and.reduce(
                [(s <= NEG_THRESH).all(axis=1) for s in subs])
            q0 = 0
            while q0 < LQC and dead_q[q0]:
                q0 += 1
            q0 &= ~127  # keep alignment coarse; only full-128 steps trimmed
            trim[(j, i)] = q0
            if not all((s == 0.0).all() for s in subs):
                band_list.append((j, i))
        live[j] = lv
    return live, band_list, trim


def _build_graph(live_key, band_key, trim_key, live_k):
    key = (live_key, band_key, trim_key, tuple(live_k))
    if key in _graph_cache:
        return _graph_cache[key]

    live = {j: list(lv) for j, lv in live_key}
    band_list = list(band_key)
    trim = dict(trim_key)
    band_idx = {ji: n for n, ji in enumerate(band_list)}
    nb = max(1, len(band_list))
    nt = len(live_k)              # live k tiles
    Lk = nt * KT
    pos_of = {i: t for t, i in enumerate(live_k)}
    NSK = (Lk + 511) // 512       # kT column sub-blocks

    nc = bacc.Bacc("TRN2", target_bir_lowering=False, debug=False,
                   num_devices=N_CORES)

    # host-prelayouted inputs: [128, ...] sbuf-shaped flat rows
    qT = nc.dram_tensor("qT", [128, NKC * L], BF16, kind="ExternalInput")
    kvT = nc.dram_tensor("kvT", [128, NKC * Lk], BF16, kind="ExternalInput")
    wq = nc.dram_tensor("wq", [128, NKC * 512], BF16, kind="ExternalInput")
    # wkv: chunk k -> cols [256k:256k+128] = wk, [256k+128:256k+256] = wv
    wkv = nc.dram_tensor("wkv", [128, NKC * 256], BF16, kind="ExternalInput")
    # wo: gathered-row order (rank, pair, half, d) x own 512-col f-slab
    wo = nc.dram_tensor("wo", [128, NKC * 512], BF16, kind="ExternalInput")
    # consts_bf: cols 0:128 ones; row 0 cols 128:640 = bv4
    consts_bf = nc.dram_tensor("consts_bf", [128, 640], BF16,
                               kind="ExternalInput")
    # consts_f32: cols 0:4 bq, col 4 bk, cols 5:9 bo f-slab
    consts_f32 = nc.dram_tensor("consts_f32", [128, 21], F32,
                                kind="ExternalInput")
    band = nc.dram_tensor("band", [128, nb * LQC], BF16, kind="ExternalInput")

    ag_warm_in = nc.dram_tensor("ag_warm_in", [32, 16], BF16)
    ag_warm_out = nc.dram_tensor("ag_warm_out", [128, 16], BF16)
    ag_in = [nc.dram_tensor(f"ag_in{j}", [512, LQC], BF16)
             for j in range(NJ)]
    ag_out = [nc.dram_tensor(f"ag_out{j}", [4 * 512, LQC], BF16)
              for j in range(NJ)]
    out_ext = [nc.dram_tensor(f"out{j}", [512, LQC], BF16,
                              kind="ExternalOutput") for j in range(NJ)]
    groups = [[0, 1, 2, 3], [4, 5, 6, 7]]

    with tile.TileContext(nc) as tc:
        with tc.tile_pool(name="persist", bufs=1) as persist:
            # DMA plan (issue cost ~2.5us/DMA per queue -> few, big, ordered):
            #  sync:   consts_bf, kv pieces, consts_f32, wq, qt pieces, band, wo
            #  scalar: wkv, ats->ag_in during attn, atf gathers
            #  gpsimd: collective warmup + AG triggers
            cbf_sb = persist.tile([128, 640], BF16, tag="cbf")
            cf_sb = persist.tile([128, 21], F32, tag="cf")
            wq_sb = persist.tile([128, NKC * 512], BF16, tag="wq")
            wo_sb = persist.tile([128, NKC * 512], BF16, tag="wo")
            band_sb = persist.tile([128, nb * LQC], BF16, tag="band")
            kT_sb = persist.tile([128, Lk], BF16, tag="kT")
            v_sb = [persist.tile([128, 130], BF16, tag=f"v{t}", name=f"v{t}")
                    for t in range(nt)]
            QT_sb = [[persist.tile([128, 1024], BF16, tag=f"qt{m}{jp}",
                                   name=f"qt{m}{jp}") for jp in range(2)]
                     for m in range(NPAIR)]
            ones_sb = cbf_sb[:, 0:128]
            bv4_sb = cbf_sb[0:1, 128:640]
            bq_sb = cf_sb[:, 0:4]
            bk_sb = cf_sb[:, 4:5]
            bo_sb = cf_sb[:, 5:9]

            with tc.tile_pool(name="qtp_scope", bufs=1) as qtsc:
                wkv_sb = qtsc.tile([128, NKC * 256], BF16, tag="wkv")
                # qt staging read by the Q jp1 sub-eras inside attention
                # scope 1, so it lives at qtp_scope level.
                qtp = [[qtsc.tile([128, 8 * 1024], BF16, tag=f"qtp{jp}{h}",
                                  name=f"qtp{jp}{h}") for h in range(2)]
                       for jp in range(2)]

                nc.sync.dma_start(cbf_sb[:], consts_bf[:])
                nc.scalar.dma_start(wkv_sb[:], wkv[:])
                for t in range(nt):
                    nc.vector.tensor_copy(v_sb[t][:, 64:65], ones_sb[:, 0:1])
                    nc.vector.tensor_copy(v_sb[t][:, 129:130], ones_sb[:, 0:1])

                # ---- KV projection (kvch era-scoped; queue-ordered DMAs)
                with (
                    tc.tile_pool(name="kv_stream", bufs=1) as kvs,
                    tc.tile_pool(name="kv_psum", bufs=1, space="PSUM") as kvp,
                ):
                    # ALL critical input transfers go on the sync ring in
                    # exact consumption order -- SDMA round-robins *rings* at
                    # packet granularity, so priority only exists within one
                    # ring.
                    kvpc = [kvs.tile([128, 4 * Lk], BF16, tag=f"kvpc{p}",
                                     name=f"kvpc{p}") for p in range(4)]
                    for p in range(4):
                        nc.sync.dma_start(kvpc[p][:],
                                          kvT[:, 4 * p * Lk:4 * (p + 1) * Lk])
                    nc.sync.dma_start(cf_sb[:], consts_f32[:])
                    nc.sync.dma_start(wq_sb[:], wq[:])
                    for jp in range(2):
                        for h in range(2):
                            nc.sync.dma_start(
                                qtp[jp][h][:],
                                qT[:, (2 * jp + h) * 8192:
                                      (2 * jp + h + 1) * 8192])
                    # band/wo after the hot inputs on the same ring
                    nc.sync.dma_start(band_sb[:], band[:])
                    nc.sync.dma_start(wo_sb[:], wo[:])
                    nc.gpsimd.dma_start(ag_warm_in[:], consts_bf[0:32, 0:16])
                    nc.gpsimd.collective_compute(
                        "AllGather", mybir.AluOpType.bypass,
                        replica_groups=groups,
                        ins=[ag_warm_in[:]], outs=[ag_warm_out[:]])
                    wids = [min(512, Lk - 512 * s) for s in range(NSK)]
                    psk = [kvp.tile([128, wids[s]], F32, tag=f"psk{s}",
                                    name=f"psk{s}") for s in range(NSK)]
                    psv = [kvp.tile([128, wids[s]], F32, tag=f"psv{s}",
                                    name=f"psv{s}") for s in range(NSK)]
                    for s in range(NSK):
                        nc.tensor.matmul(psv[s][:], ones_sb[0:1, :],
                                         bv4_sb[:, 0:wids[s]], start=True,
                                         stop=False, skip_group_check=True)
                    for k in range(NKC):
                        kv_ch = kvpc[k // 4]
                        off = (k % 4) * Lk
                        for s in range(NSK):
                            nc.tensor.matmul(
                                psk[s][:], wkv_sb[:, 256 * k:256 * k + 128],
                                kv_ch[:, off + 512 * s:
                                      off + 512 * s + wids[s]],
                                start=(k == 0), stop=(k == NKC - 1))
                        for t in range(nt):
                            s, col = t // 4, t % 4
                            nc.tensor.matmul(
                                psv[s][:, 128 * col:128 * (col + 1)],
                                kv_ch[:, off + 128 * t:off + 128 * (t + 1)],
                                wkv_sb[:, 256 * k + 128:256 * (k + 1)],
                                start=False, stop=(k == NKC - 1),
                                skip_group_check=True)
                    for s in range(NSK):
                        nc.scalar.activation(
                            kT_sb[:, 512 * s:512 * s + wids[s]],
                            psk[s][:], AF.Identity, bias=bk_sb[:])
                    for t in range(nt):
                        s, col = t // 4, t % 4
                        nc.scalar.copy(v_sb[t][:, 0:64],
                                       psv[s][:, 128 * col:128 * col + 64])
                        nc.vector.tensor_copy(
                            v_sb[t][:, 65:129],
                            psv[s][:, 128 * col + 64:128 * (col + 1)])

                # ---- Q projection from resident qT (jp-major layout).
                # jp=1 is interleaved into attention chunk 0 (scope 1 below).
                with tc.tile_pool(name="q_psum", bufs=1, space="PSUM") as qp:
                    for jp in range(1):
                        psq = [qp.tile([128, 512], F32, tag=f"psq{n}",
                                       name=f"psq{n}") for n in range(8)]
                        for k in range(NKC):
                            qch = qtp[jp][k // 8]
                            off = (k % 8) * 1024
                            for m in range(NPAIR):
                                for jj in range(2):
                                    nc.tensor.matmul(
                                        psq[4 * jj + m][:],
                                        wq_sb[:, 512 * k + 128 * m:
                                                 512 * k + 128 * (m + 1)],
                                        qch[:, off + 512 * jj:
                                            off + 512 * (jj + 1)],
                                        start=(k == 0), stop=(k == NKC - 1))
                        for jj in range(2):
                            for m in range(NPAIR):
                                nc.scalar.activation(
                                    QT_sb[m][jp][:, 512 * jj:512 * (jj + 1)],
                                    psq[4 * jj + m][:], AF.Identity,
                                    bias=bq_sb[:, m:m + 1])

                # ---- Attention scope 1: chunk 0 with Q jp=1 sub-eras.
                P = {}
                # two-stage deferred normalize for scope 2: stage A broadcasts
                # the reciprocal row via a PE ones-matmul into PSUM one block
                # after its recips; stage B (muls + at->DRAM exports) runs one
                # block after that, so no PE-feeding queue ever waits on a
                # laggy cross-engine producer.
                norm_bc = []   # (ra, rb, ua, ub, dst, row0)
                norm_mul = []  # (ua, ub, rba, rbb, dst, row0)

                def flush_norm():
                    if norm_mul:
                        ua, ub, rba, rbb, dst, row0 = norm_mul.pop(0)
                        at_a = P["at_pool"].tile([64, 512], BF16, tag="at_a")
                        at_b = P["at_pool"].tile([64, 512], BF16, tag="at_b")
                        nc.vector.tensor_mul(at_a[:], ua[0:64, :], rba[:])
                        nc.vector.tensor_mul(at_b[:], ub[0:64, :], rbb[:])
                        nc.gpsimd.dma_start(dst[row0:row0 + 64, :], at_a[:])
                        nc.gpsimd.dma_start(dst[row0 + 64:row0 + 128, :],
                                            at_b[:])
                    if norm_bc:
                        ra, rb, ua, ub, dst, row0 = norm_bc.pop(0)
                        rba = P["bc_psum"].tile([64, 512], F32, tag="rba")
                        rbb = P["bc_psum"].tile([64, 512], F32, tag="rbb")
                        nc.tensor.matmul(rba[:], ones_sb[0:1, 0:64], ra[:],
                                         start=True, stop=True,
                                         skip_group_check=True)
                        nc.tensor.matmul(rbb[:], ones_sb[0:1, 0:64], rb[:],
                                         start=True, stop=True,
                                         skip_group_check=True)
                        norm_mul.append((ua, ub, rba, rbb, dst, row0))

                def drain_norm():
                    while norm_mul or norm_bc:
                        flush_norm()

                def attn_block(j, pr, dst, dst_row0):
                    """Compute attention for (chunk j, pair pr); stage the two
                    normalized 64-row halves for export to DRAM tensor dst at
                    rows dst_row0 / dst_row0+64 (finished by the next
                    flush_norm)."""
                    lv = live[j]
                    nlast = len(lv) - 1
                    pva = P["pv_psum"].tile([65, 512], F32, tag="pva")
                    pvb = P["pv_psum"].tile([65, 512], F32, tag="pvb")
                    pts = []
                    for n, i in enumerate(lv):
                        t = pos_of[i]
                        q0 = trim.get((j, i), 0)
                        qt_t = QT_sb[pr][j // 2]
                        qoff = 512 * (j % 2)
                        ps = P["qk_psum"].tile([128, 1024], F32, tag="qk")
                        nc.tensor.matmul(
                            ps[:, q0:512],
                            kT_sb[0:64, 128 * t:128 * (t + 1)],
                            qt_t[0:64, qoff + q0:qoff + 512],
                            start=True, stop=True, skip_group_check=True)
                        nc.tensor.matmul(
                            ps[:, 512 + q0:1024],
                            kT_sb[64:128, 128 * t:128 * (t + 1)],
                            qt_t[64:128, qoff + q0:qoff + 512],
                            start=True, stop=True, skip_group_check=True)
                        pt = P["pt_pool"].tile([128, 1024], BF16, tag="pt")
                        if q0 == 0:
                            nc.scalar.activation(pt[:], ps[:], AF.Exp)
                        else:
                            nc.scalar.activation(pt[:, q0:512],
                                                 ps[:, q0:512], AF.Exp)
                            nc.scalar.activation(pt[:, 512 + q0:1024],
                                                 ps[:, 512 + q0:1024], AF.Exp)
                        if (j, i) in band_idx:
                            bcol = band_idx[(j, i)] * LQC
                            nc.vector.tensor_mul(
                                pt[:, q0:512], pt[:, q0:512],
                                band_sb[:, bcol + q0:bcol + 512])
                            nc.vector.tensor_mul(
                                pt[:, 512 + q0:1024], pt[:, 512 + q0:1024],
                                band_sb[:, bcol + q0:bcol + 512])
                        pts.append((pt, q0))
                        if n == 2:
                            # previous block's normalize finish lands here,
                            # giving its broadcast a full block of slack
                            flush_norm()

                        def pv_step(m):
                            ptp, q0p = pts[m]
                            tp = pos_of[lv[m]]
                            nc.tensor.matmul(
                                pva[:, q0p:512], v_sb[tp][:, 0:65],
                                ptp[:, q0p:512], start=(m == 0),
                                stop=(m == nlast), skip_group_check=True)
                            nc.tensor.matmul(
                                pvb[:, q0p:512], v_sb[tp][:, 65:130],
                                ptp[:, 512 + q0p:1024], start=(m == 0),
                                stop=(m == nlast), skip_group_check=True)

                        # software pipeline: PV lags QK/exp by two steps so
                        # the PE never catches up with exp+band (pt bufs=3)
                        if n > 1:
                            pv_step(n - 2)
                    pv_step(nlast - 1)
                    pv_step(nlast)
                    # unload + normalize (vector only; scalar stays pure exp)
                    ua = P["ua_pool"].tile([65, 512], F32, tag="ua")
                    ub = P["ua_pool"].tile([65, 512], F32, tag="ub")
                    nc.vector.tensor_copy(ua[:], pva[:])
                    nc.vector.tensor_copy(ub[:], pvb[:])
                    # den rows to partition 0 (DVE cannot partition-shift
                    # SBUF->SBUF; PSUM row reads to p0 are fine)
                    rsa = P["rc_pool"].tile([1, 512], F32, tag="rsa")
                    rsb = P["rc_pool"].tile([1, 512], F32, tag="rsb")
                    nc.vector.tensor_copy(rsa[:], pva[64:65, :])
                    nc.vector.tensor_copy(rsb[:], pvb[64:65, :])
                    ra = P["rc_pool"].tile([1, 512], F32, tag="ra")
                    rb = P["rc_pool"].tile([1, 512], F32, tag="rb")
                    nc.vector.reciprocal_approx_fast(out=ra[:], in_=rsa[:])
                    nc.vector.reciprocal_approx_fast(out=rb[:], in_=rsb[:])
                    if P.get("defer"):
                        # bf16 copies feed the PE ones-matmul broadcast
                        rah = P["rc_pool"].tile([1, 512], BF16, tag="rah")
                        rbh = P["rc_pool"].tile([1, 512], BF16, tag="rbh")
                        nc.vector.tensor_copy(rah[:], ra[:])
                        nc.vector.tensor_copy(rbh[:], rb[:])
                        norm_bc.append((rah, rbh, ua, ub, dst, dst_row0))
                    else:
                        # chunk 0: no collective in flight, gpsimd broadcast
                        # + immediate finish is safe and cheapest
                        rba = P["rb_pool"].tile([64, 512], F32, tag="rba")
                        rbb = P["rb_pool"].tile([64, 512], F32, tag="rbb")
                        nc.gpsimd.partition_broadcast(rba[:], ra[:])
                        nc.gpsimd.partition_broadcast(rbb[:], rb[:])
                        at_a = P["at_pool"].tile([64, 512], BF16, tag="at_a")
                        at_b = P["at_pool"].tile([64, 512], BF16, tag="at_b")
                        nc.vector.tensor_mul(at_a[:], ua[0:64, :], rba[:])
                        nc.vector.tensor_mul(at_b[:], ub[0:64, :], rbb[:])
                        nc.gpsimd.dma_start(
                            dst[dst_row0:dst_row0 + 64, :], at_a[:])
                        nc.gpsimd.dma_start(
                            dst[dst_row0 + 64:dst_row0 + 128, :], at_b[:])

                def q_sub_era(s, qsub):
                    jj, mp = s // 2, s % 2
                    psq = qsub.tile([128, 1024], F32, tag="qsub")
                    for k in range(NKC):
                        qch = qtp[1][k // 8]
                        off = (k % 8) * 1024 + 512 * jj
                        for mi in range(2):
                            m = 2 * mp + mi
                            nc.tensor.matmul(
                                psq[:, 512 * mi:512 * (mi + 1)],
                                wq_sb[:, 512 * k + 128 * m:
                                         512 * k + 128 * (m + 1)],
                                qch[:, off:off + 512],
                                start=(k == 0), stop=(k == NKC - 1))
                    for mi in range(2):
                        m = 2 * mp + mi
                        nc.scalar.activation(
                            QT_sb[m][1][:, 512 * jj:512 * (jj + 1)],
                            psq[:, 512 * mi:512 * (mi + 1)], AF.Identity,
                            bias=bq_sb[:, m:m + 1])

                with (
                    tc.tile_pool(name="pt1", bufs=3) as _pt1,
                    tc.tile_pool(name="ua1", bufs=2) as _ua1,
                    tc.tile_pool(name="rc1", bufs=2) as _rc1,
                    tc.tile_pool(name="rb1", bufs=2) as _rb1,
                    tc.tile_pool(name="at1", bufs=4) as _at1,
                    tc.tile_pool(name="qk1", bufs=2, space="PSUM") as _qk1,
                    tc.tile_pool(name="pv1", bufs=1, space="PSUM") as _pv1,
                    tc.tile_pool(name="qsub", bufs=1, space="PSUM") as _qs,
                ):
                    P.update(pt_pool=_pt1, ua_pool=_ua1, rc_pool=_rc1,
                             rb_pool=_rb1, at_pool=_at1,
                             qk_psum=_qk1, pv_psum=_pv1, defer=False)
                    for pr in range(NPAIR):
                        attn_block(0, pr, ag_in[0], 128 * pr)
                        q_sub_era(pr, _qs)
                    nc.gpsimd.collective_compute(
                        "AllGather", mybir.AluOpType.bypass,
                        replica_groups=groups,
                        ins=[ag_in[0][:]], outs=[ag_out[0][:]])

            # ---- Scope 2: chunks 1..3 + AGs, then deferred oproj tail.
            # atf holds all four gathered chunks (read only in the tail).
            with tc.tile_pool(name="atf", bufs=4) as _atf:

                def gather_to_sbuf(src_dram, a, tag):
                    # gpsimd: queued right behind the AllGather that produces
                    # src_dram, so it issues the moment the AG completes
                    t = _atf.tile([128, a * 512], BF16, tag=tag)
                    dst_ap = t[:].rearrange("p (a c) -> p a c", a=a)
                    src_ap = src_dram[:].rearrange("(a p) c -> p a c", p=128)
                    nc.gpsimd.dma_start(dst_ap, src_ap)
                    return t

                atf = [None] * NJ

                with (
                    tc.tile_pool(name="pt2", bufs=3) as _pt2,
                    tc.tile_pool(name="ua2", bufs=3) as _ua2,
                    tc.tile_pool(name="rc2", bufs=2) as _rc2,
                    tc.tile_pool(name="at2", bufs=6) as _at2,
                    tc.tile_pool(name="qk2", bufs=2, space="PSUM") as _qk2,
                    tc.tile_pool(name="pv2", bufs=1, space="PSUM") as _pv2,
                    tc.tile_pool(name="bc2", bufs=1, space="PSUM") as _bc2,
                ):
                    P.update(pt_pool=_pt2, ua_pool=_ua2, rc_pool=_rc2,
                             at_pool=_at2, bc_psum=_bc2,
                             qk_psum=_qk2, pv_psum=_pv2, defer=True)
                    atf[0] = gather_to_sbuf(ag_out[0], 16, "atf")

                    for j in range(1, NJ):
                        for pr in range(NPAIR):
                            attn_block(j, pr, ag_in[j], 128 * pr)
                        drain_norm()
                        nc.gpsimd.collective_compute(
                            "AllGather", mybir.AluOpType.bypass,
                            replica_groups=groups,
                            ins=[ag_in[j][:]], outs=[ag_out[j][:]])
                        atf[j] = gather_to_sbuf(ag_out[j], 16, "atf")

                # ---- Deferred o-projection tail: local f-slab per chunk
                # (own PSUM scope; the attention pools above are closed).
                with (
                    tc.tile_pool(name="osb2", bufs=2) as _osb2,
                    tc.tile_pool(name="o_ps", bufs=2, space="PSUM") as _ops,
                ):
                    for j in range(NJ):
                        osb = _osb2.tile([128, 4 * 512], BF16, tag="osb")
                        for fb in range(4):
                            pso = _ops.tile([128, 512], F32, tag="pso")
                            for dc in range(16):
                                mv = atf[j][:, 512 * dc:512 * (dc + 1)]
                                wcol = 512 * dc + 128 * fb
                                nc.tensor.matmul(
                                    pso[:], wo_sb[:, wcol:wcol + 128], mv,
                                    start=(dc == 0), stop=(dc == 15),
                                    skip_group_check=True)
                            sl = osb[:, 512 * fb:512 * (fb + 1)]
                            if fb % 2 == 0:
                                nc.scalar.activation(sl, pso[:], AF.Identity,
                                                     bias=bo_sb[:, fb:fb + 1])
                            else:
                                nc.vector.tensor_scalar_add(
                                    sl, pso[:], bo_sb[:, fb:fb + 1])
                        dst_ap = out_ext[j][:].rearrange("(a p) c -> p a c",
                                                         p=128)
                        src_ap = osb[:].rearrange("p (a c) -> p a c", a=4)
                        nc.sync.dma_start(dst_ap, src_ap)

    nc.compile()
    _graph_cache[key] = nc
    return nc


def _prelayout(a, width):
    """[NKC*128, width] row-major -> [128, NKC*width] sbuf layout."""
    return np.ascontiguousarray(
        a.reshape(NKC, 128, width).transpose(1, 0, 2).reshape(128, NKC * width))


def kernel(query, kv, Wq, bq, Wkv, bkv, Wo, bo, attn_mask, key_padding_mask):
    global last_results
    query = np.asarray(query, np.float32)
    kv = np.asarray(kv, np.float32)
    Wq = np.asarray(Wq, np.float32)
    bq = np.asarray(bq, np.float32)
    Wkv = np.asarray(Wkv, np.float32)
    bkv = np.asarray(bkv, np.float32)
    Wo = np.asarray(Wo, np.float32)
    bo = np.asarray(bo, np.float32)
    attn_mask = np.asarray(attn_mask, np.float32)
    kpm = np.asarray(key_padding_mask)

    eff = [attn_mask + np.where(kpm[b], np.float32(-1e9),
                                np.float32(0.0))[None, :]
           for b in range(B)]
    live, band_list, trim = _classify_blocks(eff)
    live_k = sorted({i for lv in live.values() for i in lv})
    live_key = tuple((j, tuple(lv)) for j, lv in sorted(live.items()))
    band_key = tuple(band_list)
    trim_key = tuple(sorted(trim.items()))

    nc = _build_graph(live_key, band_key, trim_key, live_k)

    nt = len(live_k)
    Lk = nt * KT

    # Host-side shard prep. qT jp-major: col = jp*16*1024 + k*1024 + c
    qTh = [np.ascontiguousarray(
        query[b].T.astype(NPBF16).reshape(NKC, 128, 2, 1024)
        .transpose(2, 1, 0, 3).reshape(2, 128, NKC * 1024)
        .transpose(1, 0, 2).reshape(128, NKC * L)) for b in range(B)]
    kvTsel = [np.ascontiguousarray(
        kv[b].T.astype(NPBF16)
        .reshape(HID, NI, KT)[:, live_k, :].reshape(HID, Lk)) for b in range(B)]
    kvTh = [_prelayout(k_, Lk) for k_ in kvTsel]
    nb = max(1, len(band_list))
    bandh = []
    with np.errstate(over="ignore", under="ignore"):
        for b in range(B):
            if band_list:
                bandh.append(np.ascontiguousarray(np.concatenate(
                    [np.exp(eff[b][j * LQC:(j + 1) * LQC,
                                   i * KT:(i + 1) * KT].T)
                     for (j, i) in band_list], axis=1).astype(NPBF16)))
            else:
                bandh.append(np.zeros((KT, nb * LQC), NPBF16))
    ones_h = np.ones((128, 128), NPBF16)

    Wq_h = Wq.reshape(HID, NH, D)
    bq_h = bq.reshape(NH, D)

    # gathered-row permutation: global row g = 512*rank + 128*pair + 64*e + d
    g = np.arange(HID)
    head_of_g = 8 * (g // 512) + (g % 512) // 128 + 4 * ((g % 128) // 64)
    row_of_g = head_of_g * D + (g % 64)

    in_maps = []
    for c in range(N_CORES):
        b, r = c // TPR, c % TPR
        heads_q = [8 * r + pr + 4 * e for pr in range(NPAIR) for e in range(2)]
        wq_c = _prelayout(
            (Wq_h[:, heads_q, :].reshape(HID, 512) * SCALE).astype(NPBF16),
            512)
        bq_c = (bq_h[heads_q].reshape(512) * SCALE).reshape(4, 128).T
        wk_c = Wkv[:, 128 * r:128 * (r + 1)].astype(NPBF16)
        bk_c = bkv[128 * r:128 * (r + 1)]
        wv_c = Wkv[:, 512 + 128 * r:512 + 128 * (r + 1)].astype(NPBF16)
        # wkv interleave: chunk k -> [wk_k | wv_k]
        wkv_c = np.ascontiguousarray(np.concatenate(
            [np.concatenate([wk_c.reshape(NKC, 128, 128)[k],
                             wv_c.reshape(NKC, 128, 128)[k]], axis=1)
             for k in range(NKC)], axis=1))  # [128, NKC*256]
        bv_c = bkv[512 + 128 * r:512 + 128 * (r + 1)]
        # wo: gathered-row order x own f-slab columns
        wo_c = _prelayout(
            Wo[row_of_g, 512 * r:512 * (r + 1)].astype(NPBF16), 512)
        cbf = np.zeros((128, 640), NPBF16)
        cbf[:, 0:128] = ones_h
        cbf[0, 128:640] = np.tile(bv_c, 4).astype(NPBF16)
        cf = np.zeros((128, 21), np.float32)
        cf[:, 0:4] = bq_c
        cf[:, 4] = bk_c
        cf[:, 5:9] = bo[512 * r:512 * (r + 1)].reshape(4, 128).T
        in_maps.append({
            "qT": qTh[b], "kvT": kvTh[b],
            "wq": wq_c, "wkv": wkv_c, "wo": wo_c,
            "consts_bf": np.ascontiguousarray(cbf),
            "consts_f32": np.ascontiguousarray(cf),
            "band": bandh[b],
        })

    last_results = run_bass_kernel_spmd(nc, in_maps,
                                        core_ids=list(range(N_CORES)))

    out = np.empty((B, L, HID), np.float32)
    for c in range(N_CORES):
        b, r = c // TPR, c % TPR
        for j in range(NJ):
            out[b, 512 * j:512 * (j + 1), 512 * r:512 * (r + 1)] = \
                last_results.results[c][f"out{j}"].T.astype(np.float32)
    return out


# revision 16
# speedup vs baseline: 1.0891x; 1.0044x over previous
"""GQA attention (B=2, L=2048, HID=2048, 32 Q heads / 8 KV heads) on 8 TRN2 cores.

Sharding: data-parallel on batch (2) x tensor-parallel on heads (4).
Core c: batch b = c//4, TP rank r = c%4 owns q heads {8r..8r+7} (whole GQA
groups: kv heads 2r, 2r+1). bf16 TensorEngine compute, fp32 PSUM, fp32
softmax statistics. Per-core pipeline:
  1. KV proj (streamed kvT pieces, padded k-tiles skipped entirely):
     kT [128, Lk] and per-tile v [128, 130] (+ones cols for the softmax
     denominator ride-along).
  2. Q proj from SBUF-resident qT: QT[pr] = [128, L] bf16, head-pair rows.
  3. Attention per (j, pr): software-pipelined QK -> exp -> (band mul) -> PV
     with the QK of step i+1 emitted before PV of step i so the PE never
     stalls behind the scalar-engine exp. Diagonal band tiles are
     column-trimmed (leading fully-masked q columns skipped in exp/PV/QK).
     Normalized attn halves are DMA'd straight to the AllGather input
     ag_in[j] in DRAM (no local output projection of own heads).
  4. Output side in AllGather form: ag_in[j] [512,512] -> AllGather over the
     4-core TP group -> ag_out[j] [2048, 512] (every core gets all heads'
     attention for the chunk). The o-projection is FULLY DEFERRED: after the
     last attention chunk the PE runs oproj for all 4 chunks back-to-back
     (local f-slab [512 f] x [512 q] per chunk, contraction over all 2048 d
     from SBUF-resident gathered atf tiles), hiding the tail AllGather under
     deferred compute. Chunk 3's AllGather is split by head-pair halves so
     the first half is triggered mid-attention.
Host assembles [2, 2048, 2048] f32 from per-core [4][512, 512] bf16 slabs.

Mask handling is input-driven: blocks are classified all-masked (skipped),
all-zero (no mask op), or band (exp(mask) multiplied into exp(scores));
band tiles additionally get a leading-dead-column trim q0.
"""

import numpy as np
import ml_dtypes
import concourse.bass as bass
import concourse.mybir as mybir
import concourse.tile as tile
from concourse import bacc
from concourse.bass_utils import run_bass_kernel_spmd

F32 = mybir.dt.float32
BF16 = mybir.dt.bfloat16
AF = mybir.ActivationFunctionType
NPBF16 = ml_dtypes.bfloat16

B, L, HID = 2, 2048, 2048
NH, D, NKV = 32, 64, 8
SCALE = 0.125
N_CORES = 8
TPR = 4          # TP ranks per batch group
NPAIR = 4        # head pairs per core
LQC = 512        # q chunk (PSUM-bank sized)
NJ = L // LQC    # 4
KT = 128         # k-position tile
NI = L // KT     # 16
NKC = HID // 128  # 16 contraction chunks
NEG_THRESH = -1.0e8

_graph_cache = {}
last_results = None  # BassKernelResults of the most recent run (for test harness)


def _classify_blocks(eff_masks):
    """eff_masks: list of B arrays [L, L] (q, k). Returns (live, band_list,
    trim) where live[j] lists live k-tiles for q-chunk j, band_list orders
    blocks needing explicit mask values, and trim[(j, i)] is the count of
    leading q-columns of the transposed block that are fully masked."""
    live = {}
    band_list = []
    trim = {}
    for j in range(NJ):
        lv = []
        for i in range(NI):
            subs = [m[j * LQC:(j + 1) * LQC, i * KT:(i + 1) * KT] for m in eff_masks]
            if all((s <= NEG_THRESH).all() for s in subs):
                continue  # fully masked in every batch
            lv.append(i)
            # leading q-columns (rows of the [LQC, KT] block) dead in all b
            dead_q = np.logical_and.reduce(
                [(s <= NEG_THRESH).all(axis=1) for s in subs])
            q0 = 0
            while q0 < LQC and dead_q[q0]:
                q0 += 1
            q0 &= ~127  # keep alignment coarse; only full-128 steps trimmed
            trim[(j, i)] = q0
            if not all((s == 0.0).all() for s in subs):
                band_list.append((j, i))
        live[j] = lv
    return live, band_list, trim


def _build_graph(live_key, band_key, trim_key, live_k):
    key = (live_key, band_key, trim_key, tuple(live_k))
    if key in _graph_cache:
        return _graph_cache[key]

    live = {j: list(lv) for j, lv in live_key}
    band_list = list(band_key)
    trim = dict(trim_key)
    band_idx = {ji: n for n, ji in enumerate(band_list)}
    nb = max(1, len(band_list))
    nt = len(live_k)              # live k tiles
    Lk = nt * KT
    pos_of = {i: t for t, i in enumerate(live_k)}
    NSK = (Lk + 511) // 512       # kT column sub-blocks

    nc = bacc.Bacc("TRN2", target_bir_lowering=False, debug=False,
                   num_devices=N_CORES)

    # host-prelayouted inputs: [128, ...] sbuf-shaped flat rows
    qT = nc.dram_tensor("qT", [128, NKC * L], BF16, kind="ExternalInput")
    kvT = nc.dram_tensor("kvT", [128, NKC * Lk], BF16, kind="ExternalInput")
    wq = nc.dram_tensor("wq", [128, NKC * 512], BF16, kind="ExternalInput")
    # wkv: chunk k -> cols [256k:256k+128] = wk, [256k+128:256k+256] = wv
    wkv = nc.dram_tensor("wkv", [128, NKC * 256], BF16, kind="ExternalInput")
    # wo: gathered-row order (rank, pair, half, d) x own 512-col f-slab
    wo = nc.dram_tensor("wo", [128, NKC * 512], BF16, kind="ExternalInput")
    # consts_bf: cols 0:128 ones; row 0 cols 128:640 = bv4
    consts_bf = nc.dram_tensor("consts_bf", [128, 640], BF16,
                               kind="ExternalInput")
    # consts_f32: cols 0:4 bq, col 4 bk, cols 5:9 bo f-slab
    consts_f32 = nc.dram_tensor("consts_f32", [128, 21], F32,
                                kind="ExternalInput")
    band = nc.dram_tensor("band", [128, nb * LQC], BF16, kind="ExternalInput")

    ag_warm_in = nc.dram_tensor("ag_warm_in", [32, 16], BF16)
    ag_warm_out = nc.dram_tensor("ag_warm_out", [128, 16], BF16)
    ag_in = [nc.dram_tensor(f"ag_in{j}", [512, LQC], BF16)
             for j in range(NJ)]
    ag_out = [nc.dram_tensor(f"ag_out{j}", [4 * 512, LQC], BF16)
              for j in range(NJ)]
    out_ext = [nc.dram_tensor(f"out{j}", [512, LQC], BF16,
                              kind="ExternalOutput") for j in range(NJ)]
    groups = [[0, 1, 2, 3], [4, 5, 6, 7]]

    with tile.TileContext(nc) as tc:
        with tc.tile_pool(name="persist", bufs=1) as persist:
            # DMA plan (issue cost ~2.5us/DMA per queue -> few, big, ordered):
            #  sync:   consts_bf, kv pieces, consts_f32, wq, qt pieces, band, wo
            #  scalar: wkv, ats->ag_in during attn, atf gathers
            #  gpsimd: collective warmup + AG triggers
            cbf_sb = persist.tile([128, 640], BF16, tag="cbf")
            cf_sb = persist.tile([128, 21], F32, tag="cf")
            wq_sb = persist.tile([128, NKC * 512], BF16, tag="wq")
            wo_sb = persist.tile([128, NKC * 512], BF16, tag="wo")
            band_sb = persist.tile([128, nb * LQC], BF16, tag="band")
            kT_sb = persist.tile([128, Lk], BF16, tag="kT")
            v_sb = [persist.tile([128, 130], BF16, tag=f"v{t}", name=f"v{t}")
                    for t in range(nt)]
            QT_sb = [[persist.tile([128, 1024], BF16, tag=f"qt{m}{jp}",
                                   name=f"qt{m}{jp}") for jp in range(2)]
                     for m in range(NPAIR)]
            ones_sb = cbf_sb[:, 0:128]
            bv4_sb = cbf_sb[0:1, 128:640]
            bq_sb = cf_sb[:, 0:4]
            bk_sb = cf_sb[:, 4:5]
            bo_sb = cf_sb[:, 5:9]

            with tc.tile_pool(name="qtp_scope", bufs=1) as qtsc:
                wkv_sb = qtsc.tile([128, NKC * 256], BF16, tag="wkv")
                # qt staging read by the Q jp1 sub-eras inside attention
                # scope 1, so it lives at qtp_scope level.
                qtp = [[qtsc.tile([128, 8 * 1024], BF16, tag=f"qtp{jp}{h}",
                                  name=f"qtp{jp}{h}") for h in range(2)]
                       for jp in range(2)]

                nc.sync.dma_start(cbf_sb[:], consts_bf[:])
                nc.scalar.dma_start(wkv_sb[:], wkv[:])
                for t in range(nt):
                    nc.vector.tensor_copy(v_sb[t][:, 64:65], ones_sb[:, 0:1])
                    nc.vector.tensor_copy(v_sb[t][:, 129:130], ones_sb[:, 0:1])

                # ---- KV projection (kvch era-scoped; queue-ordered DMAs)
                with (
                    tc.tile_pool(name="kv_stream", bufs=1) as kvs,
                    tc.tile_pool(name="kv_psum", bufs=1, space="PSUM") as kvp,
                ):
                    # ALL critical input transfers go on the sync ring in
                    # exact consumption order -- SDMA round-robins *rings* at
                    # packet granularity, so priority only exists within one
                    # ring.
                    # first piece halved so the first matmul starts sooner
                    piece_of = [0, 0, 1, 1] + [2 + (k - 4) // 4
                                               for k in range(4, NKC)]
                    piece_off = [0, 2, 4, 8, 12]
                    piece_len = [2, 2, 4, 4, 4]
                    kvpc = [kvs.tile([128, piece_len[p] * Lk], BF16,
                                     tag=f"kvpc{p}", name=f"kvpc{p}")
                            for p in range(5)]
                    for p in range(5):
                        nc.sync.dma_start(
                            kvpc[p][:],
                            kvT[:, piece_off[p] * Lk:
                                (piece_off[p] + piece_len[p]) * Lk])
                    nc.sync.dma_start(cf_sb[:], consts_f32[:])
                    nc.sync.dma_start(wq_sb[:], wq[:])
                    for jp in range(2):
                        for h in range(2):
                            nc.sync.dma_start(
                                qtp[jp][h][:],
                                qT[:, (2 * jp + h) * 8192:
                                      (2 * jp + h + 1) * 8192])
                    # band/wo after the hot inputs on the same ring
                    nc.sync.dma_start(band_sb[:], band[:])
                    nc.sync.dma_start(wo_sb[:], wo[:])
                    nc.gpsimd.dma_start(ag_warm_in[:], consts_bf[0:32, 0:16])
                    nc.gpsimd.collective_compute(
                        "AllGather", mybir.AluOpType.bypass,
                        replica_groups=groups,
                        ins=[ag_warm_in[:]], outs=[ag_warm_out[:]])
                    wids = [min(512, Lk - 512 * s) for s in range(NSK)]
                    psk = [kvp.tile([128, wids[s]], F32, tag=f"psk{s}",
                                    name=f"psk{s}") for s in range(NSK)]
                    psv = [kvp.tile([128, wids[s]], F32, tag=f"psv{s}",
                                    name=f"psv{s}") for s in range(NSK)]
                    for s in range(NSK):
                        nc.tensor.matmul(psv[s][:], ones_sb[0:1, :],
                                         bv4_sb[:, 0:wids[s]], start=True,
                                         stop=False, skip_group_check=True)
                    for k in range(NKC):
                        kv_ch = kvpc[piece_of[k]]
                        off = (k - piece_off[piece_of[k]]) * Lk
                        for s in range(NSK):
                            nc.tensor.matmul(
                                psk[s][:], wkv_sb[:, 256 * k:256 * k + 128],
                                kv_ch[:, off + 512 * s:
                                      off + 512 * s + wids[s]],
                                start=(k == 0), stop=(k == NKC - 1))
                        for t in range(nt):
                            s, col = t // 4, t % 4
                            nc.tensor.matmul(
                                psv[s][:, 128 * col:128 * (col + 1)],
                                kv_ch[:, off + 128 * t:off + 128 * (t + 1)],
                                wkv_sb[:, 256 * k + 128:256 * (k + 1)],
                                start=False, stop=(k == NKC - 1),
                                skip_group_check=True)
                    for s in range(NSK):
                        nc.scalar.activation(
                            kT_sb[:, 512 * s:512 * s + wids[s]],
                            psk[s][:], AF.Identity, bias=bk_sb[:])
                    for t in range(nt):
                        s, col = t // 4, t % 4
                        nc.scalar.copy(v_sb[t][:, 0:64],
                                       psv[s][:, 128 * col:128 * col + 64])
                        nc.vector.tensor_copy(
                            v_sb[t][:, 65:129],
                            psv[s][:, 128 * col + 64:128 * (col + 1)])

                # ---- Q projection from resident qT (jp-major layout).
                # jp=1 is interleaved into attention chunk 0 (scope 1 below).
                with tc.tile_pool(name="q_psum", bufs=1, space="PSUM") as qp:
                    for jp in range(1):
                        psq = [qp.tile([128, 512], F32, tag=f"psq{n}",
                                       name=f"psq{n}") for n in range(8)]
                        for k in range(NKC):
                            qch = qtp[jp][k // 8]
                            off = (k % 8) * 1024
                            for m in range(NPAIR):
                                for jj in range(2):
                                    nc.tensor.matmul(
                                        psq[4 * jj + m][:],
                                        wq_sb[:, 512 * k + 128 * m:
                                                 512 * k + 128 * (m + 1)],
                                        qch[:, off + 512 * jj:
                                            off + 512 * (jj + 1)],
                                        start=(k == 0), stop=(k == NKC - 1))
                        for jj in range(2):
                            for m in range(NPAIR):
                                nc.scalar.activation(
                                    QT_sb[m][jp][:, 512 * jj:512 * (jj + 1)],
                                    psq[4 * jj + m][:], AF.Identity,
                                    bias=bq_sb[:, m:m + 1])

                # ---- Attention scope 1: chunk 0 with Q jp=1 sub-eras.
                P = {}
                # two-stage deferred normalize for scope 2: stage A broadcasts
                # the reciprocal row via a PE ones-matmul into PSUM one block
                # after its recips; stage B (muls + at->DRAM exports) runs one
                # block after that, so no PE-feeding queue ever waits on a
                # laggy cross-engine producer.
                norm_bc = []   # (ra, rb, ua, ub, dst, row0)
                norm_mul = []  # (ua, ub, rba, rbb, dst, row0)

                def flush_norm():
                    if norm_mul:
                        ua, ub, rba, rbb, dst, row0 = norm_mul.pop(0)
                        at_a = P["at_pool"].tile([64, 512], BF16, tag="at_a")
                        at_b = P["at_pool"].tile([64, 512], BF16, tag="at_b")
                        nc.vector.tensor_mul(at_a[:], ua[0:64, :], rba[:])
                        nc.vector.tensor_mul(at_b[:], ub[0:64, :], rbb[:])
                        nc.gpsimd.dma_start(dst[row0:row0 + 64, :], at_a[:])
                        nc.gpsimd.dma_start(dst[row0 + 64:row0 + 128, :],
                                            at_b[:])
                    if norm_bc:
                        ra, rb, ua, ub, dst, row0 = norm_bc.pop(0)
                        rba = P["bc_psum"].tile([64, 512], F32, tag="rba")
                        rbb = P["bc_psum"].tile([64, 512], F32, tag="rbb")
                        nc.tensor.matmul(rba[:], ones_sb[0:1, 0:64], ra[:],
                                         start=True, stop=True,
                                         skip_group_check=True)
                        nc.tensor.matmul(rbb[:], ones_sb[0:1, 0:64], rb[:],
                                         start=True, stop=True,
                                         skip_group_check=True)
                        norm_mul.append((ua, ub, rba, rbb, dst, row0))

                def drain_norm():
                    while norm_mul or norm_bc:
                        flush_norm()

                def attn_block(j, pr, dst, dst_row0):
                    """Compute attention for (chunk j, pair pr); stage the two
                    normalized 64-row halves for export to DRAM tensor dst at
                    rows dst_row0 / dst_row0+64 (finished by the next
                    flush_norm)."""
                    lv = live[j]
                    nlast = len(lv) - 1
                    pva = P["pv_psum"].tile([65, 512], F32, tag="pva")
                    pvb = P["pv_psum"].tile([65, 512], F32, tag="pvb")
                    pts = []
                    for n, i in enumerate(lv):
                        t = pos_of[i]
                        q0 = trim.get((j, i), 0)
                        qt_t = QT_sb[pr][j // 2]
                        qoff = 512 * (j % 2)
                        ps = P["qk_psum"].tile([128, 1024], F32, tag="qk")
                        nc.tensor.matmul(
                            ps[:, q0:512],
                            kT_sb[0:64, 128 * t:128 * (t + 1)],
                            qt_t[0:64, qoff + q0:qoff + 512],
                            start=True, stop=True, skip_group_check=True)
                        nc.tensor.matmul(
                            ps[:, 512 + q0:1024],
                            kT_sb[64:128, 128 * t:128 * (t + 1)],
                            qt_t[64:128, qoff + q0:qoff + 512],
                            start=True, stop=True, skip_group_check=True)
                        pt = P["pt_pool"].tile([128, 1024], BF16, tag="pt")
                        if q0 == 0:
                            nc.scalar.activation(pt[:], ps[:], AF.Exp)
                        else:
                            nc.scalar.activation(pt[:, q0:512],
                                                 ps[:, q0:512], AF.Exp)
                            nc.scalar.activation(pt[:, 512 + q0:1024],
                                                 ps[:, 512 + q0:1024], AF.Exp)
                        if (j, i) in band_idx:
                            bcol = band_idx[(j, i)] * LQC
                            nc.vector.tensor_mul(
                                pt[:, q0:512], pt[:, q0:512],
                                band_sb[:, bcol + q0:bcol + 512])
                            nc.vector.tensor_mul(
                                pt[:, 512 + q0:1024], pt[:, 512 + q0:1024],
                                band_sb[:, bcol + q0:bcol + 512])
                        pts.append((pt, q0))
                        if n == 2:
                            # previous block's normalize finish lands here,
                            # giving its broadcast a full block of slack
                            flush_norm()

                        def pv_step(m):
                            ptp, q0p = pts[m]
                            tp = pos_of[lv[m]]
                            nc.tensor.matmul(
                                pva[:, q0p:512], v_sb[tp][:, 0:65],
                                ptp[:, q0p:512], start=(m == 0),
                                stop=(m == nlast), skip_group_check=True)
                            nc.tensor.matmul(
                                pvb[:, q0p:512], v_sb[tp][:, 65:130],
                                ptp[:, 512 + q0p:1024], start=(m == 0),
                                stop=(m == nlast), skip_group_check=True)

                        # software pipeline: PV lags QK/exp by two steps so
                        # the PE never catches up with exp+band (pt bufs=3)
                        if n > 1:
                            pv_step(n - 2)
                    pv_step(nlast - 1)
                    pv_step(nlast)
                    # unload + normalize (vector only; scalar stays pure exp)
                    ua = P["ua_pool"].tile([65, 512], F32, tag="ua")
                    ub = P["ua_pool"].tile([65, 512], F32, tag="ub")
                    nc.vector.tensor_copy(ua[:], pva[:])
                    nc.vector.tensor_copy(ub[:], pvb[:])
                    # den rows to partition 0 (DVE cannot partition-shift
                    # SBUF->SBUF; PSUM row reads to p0 are fine)
                    rsa = P["rc_pool"].tile([1, 512], F32, tag="rsa")
                    rsb = P["rc_pool"].tile([1, 512], F32, tag="rsb")
                    nc.vector.tensor_copy(rsa[:], pva[64:65, :])
                    nc.vector.tensor_copy(rsb[:], pvb[64:65, :])
                    ra = P["rc_pool"].tile([1, 512], F32, tag="ra")
                    rb = P["rc_pool"].tile([1, 512], F32, tag="rb")
                    nc.vector.reciprocal_approx_fast(out=ra[:], in_=rsa[:])
                    nc.vector.reciprocal_approx_fast(out=rb[:], in_=rsb[:])
                    if P.get("defer"):
                        # bf16 copies feed the PE ones-matmul broadcast
                        rah = P["rc_pool"].tile([1, 512], BF16, tag="rah")
                        rbh = P["rc_pool"].tile([1, 512], BF16, tag="rbh")
                        nc.vector.tensor_copy(rah[:], ra[:])
                        nc.vector.tensor_copy(rbh[:], rb[:])
                        norm_bc.append((rah, rbh, ua, ub, dst, dst_row0))
                    else:
                        # chunk 0: no collective in flight, gpsimd broadcast
                        # + immediate finish is safe and cheapest
                        rba = P["rb_pool"].tile([64, 512], F32, tag="rba")
                        rbb = P["rb_pool"].tile([64, 512], F32, tag="rbb")
                        nc.gpsimd.partition_broadcast(rba[:], ra[:])
                        nc.gpsimd.partition_broadcast(rbb[:], rb[:])
                        at_a = P["at_pool"].tile([64, 512], BF16, tag="at_a")
                        at_b = P["at_pool"].tile([64, 512], BF16, tag="at_b")
                        nc.vector.tensor_mul(at_a[:], ua[0:64, :], rba[:])
                        nc.vector.tensor_mul(at_b[:], ub[0:64, :], rbb[:])
                        nc.gpsimd.dma_start(
                            dst[dst_row0:dst_row0 + 64, :], at_a[:])
                        nc.gpsimd.dma_start(
                            dst[dst_row0 + 64:dst_row0 + 128, :], at_b[:])

                def q_sub_era(s, qsub):
                    jj, mp = s // 2, s % 2
                    psq = qsub.tile([128, 1024], F32, tag="qsub")
                    for k in range(NKC):
                        qch = qtp[1][k // 8]
                        off = (k % 8) * 1024 + 512 * jj
                        for mi in range(2):
                            m = 2 * mp + mi
                            nc.tensor.matmul(
                                psq[:, 512 * mi:512 * (mi + 1)],
                                wq_sb[:, 512 * k + 128 * m:
                                         512 * k + 128 * (m + 1)],
                                qch[:, off:off + 512],
                                start=(k == 0), stop=(k == NKC - 1))
                    for mi in range(2):
                        m = 2 * mp + mi
                        nc.scalar.activation(
                            QT_sb[m][1][:, 512 * jj:512 * (jj + 1)],
                            psq[:, 512 * mi:512 * (mi + 1)], AF.Identity,
                            bias=bq_sb[:, m:m + 1])

                with (
                    tc.tile_pool(name="pt1", bufs=3) as _pt1,
                    tc.tile_pool(name="ua1", bufs=2) as _ua1,
                    tc.tile_pool(name="rc1", bufs=2) as _rc1,
                    tc.tile_pool(name="rb1", bufs=2) as _rb1,
                    tc.tile_pool(name="at1", bufs=4) as _at1,
                    tc.tile_pool(name="qk1", bufs=2, space="PSUM") as _qk1,
                    tc.tile_pool(name="pv1", bufs=1, space="PSUM") as _pv1,
                    tc.tile_pool(name="qsub", bufs=1, space="PSUM") as _qs,
                ):
                    P.update(pt_pool=_pt1, ua_pool=_ua1, rc_pool=_rc1,
                             rb_pool=_rb1, at_pool=_at1,
                             qk_psum=_qk1, pv_psum=_pv1, defer=False)
                    for pr in range(NPAIR):
                        attn_block(0, pr, ag_in[0], 128 * pr)
                        q_sub_era(pr, _qs)
                    nc.gpsimd.collective_compute(
                        "AllGather", mybir.AluOpType.bypass,
                        replica_groups=groups,
                        ins=[ag_in[0][:]], outs=[ag_out[0][:]])

            # ---- Scope 2: chunks 1..3 + AGs, then deferred oproj tail.
            # atf holds all four gathered chunks (read only in the tail).
            with (
                tc.tile_pool(name="atf", bufs=3) as _atf,
                tc.tile_pool(name="atf3", bufs=1) as _atf3,
            ):
                def gather_to_sbuf(pool, src_dram, a, tag):
                    # gpsimd: queued right behind the AllGather that produces
                    # src_dram, so it issues the moment the AG completes
                    t = pool.tile([128, a * 512], BF16, tag=tag)
                    dst_ap = t[:].rearrange("p (a c) -> p a c", a=a)
                    src_ap = src_dram[:].rearrange("(a p) c -> p a c", p=128)
                    nc.gpsimd.dma_start(dst_ap, src_ap)
                    return t

                atf = [None] * NJ

                with (
                    tc.tile_pool(name="pt2", bufs=3) as _pt2,
                    tc.tile_pool(name="ua2", bufs=3) as _ua2,
                    tc.tile_pool(name="rc2", bufs=2) as _rc2,
                    tc.tile_pool(name="at2", bufs=6) as _at2,
                    tc.tile_pool(name="qk2", bufs=2, space="PSUM") as _qk2,
                    tc.tile_pool(name="pv2", bufs=1, space="PSUM") as _pv2,
                    tc.tile_pool(name="bc2", bufs=1, space="PSUM") as _bc2,
                ):
                    P.update(pt_pool=_pt2, ua_pool=_ua2, rc_pool=_rc2,
                             at_pool=_at2, bc_psum=_bc2,
                             qk_psum=_qk2, pv_psum=_pv2, defer=True)
                    atf[0] = gather_to_sbuf(_atf, ag_out[0], 16, "atf")

                    atf3b = [None]
                    for j in range(1, NJ):
                        for pr in range(NPAIR):
                            attn_block(j, pr, ag_in[j], 128 * pr)
                        drain_norm()
                        nc.gpsimd.collective_compute(
                            "AllGather", mybir.AluOpType.bypass,
                            replica_groups=groups,
                            ins=[ag_in[j][:]], outs=[ag_out[j][:]])
                        if j < NJ - 1:
                            atf[j] = gather_to_sbuf(_atf, ag_out[j], 16,
                                                    "atf")
                        else:
                            # split the last gather so oproj(3)'s first half
                            # of the contraction starts as soon as possible
                            atf[j] = gather_to_sbuf(
                                _atf3, ag_out[j][0:1024, :], 8, "atf3a")
                            atf3b[0] = gather_to_sbuf(
                                _atf3, ag_out[j][1024:2048, :], 8, "atf3b")

                # ---- Deferred o-projection tail: local f-slab per chunk
                # (own PSUM scope; the attention pools above are closed).
                with (
                    tc.tile_pool(name="osb2", bufs=2) as _osb2,
                    tc.tile_pool(name="o_ps", bufs=2, space="PSUM") as _ops,
                ):
                    for j in range(NJ):
                        osb = _osb2.tile([128, 4 * 512], BF16, tag="osb")
                        for fb in range(4):
                            pso = _ops.tile([128, 512], F32, tag="pso")
                            for dc in range(16):
                                if j == NJ - 1 and dc >= 8:
                                    mv = atf3b[0][:, 512 * (dc - 8):
                                                  512 * (dc - 7)]
                                else:
                                    mv = atf[j][:, 512 * dc:512 * (dc + 1)]
                                wcol = 512 * dc + 128 * fb
                                nc.tensor.matmul(
                                    pso[:], wo_sb[:, wcol:wcol + 128], mv,
                                    start=(dc == 0), stop=(dc == 15),
                                    skip_group_check=True)
                            sl = osb[:, 512 * fb:512 * (fb + 1)]
                            if fb % 2 == 0:
                                nc.scalar.activation(sl, pso[:], AF.Identity,
                                                     bias=bo_sb[:, fb:fb + 1])
                            else:
                                nc.vector.tensor_scalar_add(
                                    sl, pso[:], bo_sb[:, fb:fb + 1])
                        dst_ap = out_ext[j][:].rearrange("(a p) c -> p a c",
                                                         p=128)
                        src_ap = osb[:].rearrange("p (a c) -> p a c", a=4)
                        nc.sync.dma_start(dst_ap, src_ap)

    nc.compile()
    _graph_cache[key] = nc
    return nc


def _prelayout(a, width):
    """[NKC*128, width] row-major -> [128, NKC*width] sbuf layout."""
    return np.ascontiguousarray(
        a.reshape(NKC, 128, width).transpose(1, 0, 2).reshape(128, NKC * width))


def kernel(query, kv, Wq, bq, Wkv, bkv, Wo, bo, attn_mask, key_padding_mask):
    global last_results
    query = np.asarray(query, np.float32)
    kv = np.asarray(kv, np.float32)
    Wq = np.asarray(Wq, np.float32)
    bq = np.asarray(bq, np.float32)
    Wkv = np.asarray(Wkv, np.float32)
    bkv = np.asarray(bkv, np.float32)
    Wo = np.asarray(Wo, np.float32)
    bo = np.asarray(bo, np.float32)
    attn_mask = np.asarray(attn_mask, np.float32)
    kpm = np.asarray(key_padding_mask)

    eff = [attn_mask + np.where(kpm[b], np.float32(-1e9),
                                np.float32(0.0))[None, :]
           for b in range(B)]
    live, band_list, trim = _classify_blocks(eff)
    live_k = sorted({i for lv in live.values() for i in lv})
    live_key = tuple((j, tuple(lv)) for j, lv in sorted(live.items()))
    band_key = tuple(band_list)
    trim_key = tuple(sorted(trim.items()))

    nc = _build_graph(live_key, band_key, trim_key, live_k)

    nt = len(live_k)
    Lk = nt * KT

    # Host-side shard prep. qT jp-major: col = jp*16*1024 + k*1024 + c
    qTh = [np.ascontiguousarray(
        query[b].T.astype(NPBF16).reshape(NKC, 128, 2, 1024)
        .transpose(2, 1, 0, 3).reshape(2, 128, NKC * 1024)
        .transpose(1, 0, 2).reshape(128, NKC * L)) for b in range(B)]
    kvTsel = [np.ascontiguousarray(
        kv[b].T.astype(NPBF16)
        .reshape(HID, NI, KT)[:, live_k, :].reshape(HID, Lk)) for b in range(B)]
    kvTh = [_prelayout(k_, Lk) for k_ in kvTsel]
    nb = max(1, len(band_list))
    bandh = []
    with np.errstate(over="ignore", under="ignore"):
        for b in range(B):
            if band_list:
                bandh.append(np.ascontiguousarray(np.concatenate(
                    [np.exp(eff[b][j * LQC:(j + 1) * LQC,
                                   i * KT:(i + 1) * KT].T)
                     for (j, i) in band_list], axis=1).astype(NPBF16)))
            else:
                bandh.append(np.zeros((KT, nb * LQC), NPBF16))
    ones_h = np.ones((128, 128), NPBF16)

    Wq_h = Wq.reshape(HID, NH, D)
    bq_h = bq.reshape(NH, D)

    # gathered-row permutation: global row g = 512*rank + 128*pair + 64*e + d
    g = np.arange(HID)
    head_of_g = 8 * (g // 512) + (g % 512) // 128 + 4 * ((g % 128) // 64)
    row_of_g = head_of_g * D + (g % 64)

    in_maps = []
    for c in range(N_CORES):
        b, r = c // TPR, c % TPR
        heads_q = [8 * r + pr + 4 * e for pr in range(NPAIR) for e in range(2)]
        wq_c = _prelayout(
            (Wq_h[:, heads_q, :].reshape(HID, 512) * SCALE).astype(NPBF16),
            512)
        bq_c = (bq_h[heads_q].reshape(512) * SCALE).reshape(4, 128).T
        wk_c = Wkv[:, 128 * r:128 * (r + 1)].astype(NPBF16)
        bk_c = bkv[128 * r:128 * (r + 1)]
        wv_c = Wkv[:, 512 + 128 * r:512 + 128 * (r + 1)].astype(NPBF16)
        # wkv interleave: chunk k -> [wk_k | wv_k]
        wkv_c = np.ascontiguousarray(np.concatenate(
            [np.concatenate([wk_c.reshape(NKC, 128, 128)[k],
                             wv_c.reshape(NKC, 128, 128)[k]], axis=1)
             for k in range(NKC)], axis=1))  # [128, NKC*256]
        bv_c = bkv[512 + 128 * r:512 + 128 * (r + 1)]
        # wo: gathered-row order x own f-slab columns
        wo_c = _prelayout(
            Wo[row_of_g, 512 * r:512 * (r + 1)].astype(NPBF16), 512)
        cbf = np.zeros((128, 640), NPBF16)
        cbf[:, 0:128] = ones_h
        cbf[0, 128:640] = np.tile(bv_c, 4).astype(NPBF16)
        cf = np.zeros((128, 21), np.float32)
        cf[:, 0:4] = bq_c
        cf[:, 4] = bk_c
        cf[:, 5:9] = bo[512 * r:512 * (r + 1)].reshape(4, 128).T
        in_maps.append({
            "qT": qTh[b], "kvT": kvTh[b],
            "wq": wq_c, "wkv": wkv_c, "wo": wo_c,
            "consts_bf": np.ascontiguousarray(cbf),
            "consts_f32": np.ascontiguousarray(cf),
            "band": bandh[b],
        })

    last_results = run_bass_kernel_spmd(nc, in_maps,
                                        core_ids=list(range(N_CORES)))

    out = np.empty((B, L, HID), np.float32)
    for c in range(N_CORES):
        b, r = c // TPR, c % TPR
        for j in range(NJ):
            out[b, 512 * j:512 * (j + 1), 512 * r:512 * (r + 1)] = \
                last_results.results[c][f"out{j}"].T.astype(np.float32)
    return out


# revision 17
# speedup vs baseline: 1.0953x; 1.0057x over previous
"""GQA attention (B=2, L=2048, HID=2048, 32 Q heads / 8 KV heads) on 8 TRN2 cores.

Sharding: data-parallel on batch (2) x tensor-parallel on heads (4).
Core c: batch b = c//4, TP rank r = c%4 owns q heads {8r..8r+7} (whole GQA
groups: kv heads 2r, 2r+1). bf16 TensorEngine compute, fp32 PSUM, fp32
softmax statistics. Per-core pipeline:
  1. KV proj (streamed kvT pieces, padded k-tiles skipped entirely):
     kT [128, Lk] and per-tile v [128, 130] (+ones cols for the softmax
     denominator ride-along).
  2. Q proj from SBUF-resident qT: QT[pr] = [128, L] bf16, head-pair rows.
  3. Attention per (j, pr): software-pipelined QK -> exp -> (band mul) -> PV
     with the QK of step i+1 emitted before PV of step i so the PE never
     stalls behind the scalar-engine exp. Diagonal band tiles are
     column-trimmed (leading fully-masked q columns skipped in exp/PV/QK).
     Normalized attn halves are DMA'd straight to the AllGather input
     ag_in[j] in DRAM (no local output projection of own heads).
  4. Output side in AllGather form: ag_in[j] [512,512] -> AllGather over the
     4-core TP group -> ag_out[j] [2048, 512] (every core gets all heads'
     attention for the chunk). The o-projection is FULLY DEFERRED: after the
     last attention chunk the PE runs oproj for all 4 chunks back-to-back
     (local f-slab [512 f] x [512 q] per chunk, contraction over all 2048 d
     from SBUF-resident gathered atf tiles), hiding the tail AllGather under
     deferred compute. Chunk 3's gather DMA is split in two so oproj(3) can
     start on the first half of the contraction sooner.
Queue discipline (the load-bearing part): scalar = exp only; vector = band
muls + psum unloads + recips + (one-block-lagged) normalize muls; gpsimd =
AllGather triggers + at->DRAM exports + atf gathers (a triggering queue is
blocked until its collective completes, so nothing PE-critical lives there);
PE broadcasts the reciprocal row via a ones[1,64] stationary matmul into
PSUM, replacing gpsimd partition_broadcast in collective-active phases.
Host assembles [2, 2048, 2048] f32 from per-core [4][512, 512] bf16 slabs.

Mask handling is input-driven: blocks are classified all-masked (skipped),
all-zero (no mask op), or band (exp(mask) multiplied into exp(scores));
band tiles additionally get a leading-dead-column trim q0.
"""

import numpy as np
import ml_dtypes
import concourse.bass as bass
import concourse.mybir as mybir
import concourse.tile as tile
from concourse import bacc
from concourse.bass_utils import run_bass_kernel_spmd

F32 = mybir.dt.float32
BF16 = mybir.dt.bfloat16
AF = mybir.ActivationFunctionType
NPBF16 = ml_dtypes.bfloat16

B, L, HID = 2, 2048, 2048
NH, D, NKV = 32, 64, 8
SCALE = 0.125
N_CORES = 8
TPR = 4          # TP ranks per batch group
NPAIR = 4        # head pairs per core
LQC = 512        # q chunk (PSUM-bank sized)
NJ = L // LQC    # 4
KT = 128         # k-position tile
NI = L // KT     # 16
NKC = HID // 128  # 16 contraction chunks
NEG_THRESH = -1.0e8

_graph_cache = {}
last_results = None  # BassKernelResults of the most recent run (for test harness)


def _classify_blocks(eff_masks):
    """eff_masks: list of B arrays [L, L] (q, k). Returns (live, band_list,
    trim) where live[j] lists live k-tiles for q-chunk j, band_list orders
    blocks needing explicit mask values, and trim[(j, i)] is the count of
    leading q-columns of the transposed block that are fully masked."""
    live = {}
    band_list = []
    trim = {}
    for j in range(NJ):
        lv = []
        for i in range(NI):
            subs = [m[j * LQC:(j + 1) * LQC, i * KT:(i + 1) * KT] for m in eff_masks]
            if all((s <= NEG_THRESH).all() for s in subs):
                continue  # fully masked in every batch
            lv.append(i)
            # leading q-columns (rows of the [LQC, KT] block) dead in all b
            dead_q = np.logical_and.reduce(
                [(s <= NEG_THRESH).all(axis=1) for s in subs])
            q0 = 0
            while q0 < LQC and dead_q[q0]:
                q0 += 1
            q0 &= ~127  # keep alignment coarse; only full-128 steps trimmed
            trim[(j, i)] = q0
            if not all((s == 0.0).all() for s in subs):
                band_list.append((j, i))
        live[j] = lv
    return live, band_list, trim


def _build_graph(live_key, band_key, trim_key, live_k):
    key = (live_key, band_key, trim_key, tuple(live_k))
    if key in _graph_cache:
        return _graph_cache[key]

    live = {j: list(lv) for j, lv in live_key}
    band_list = list(band_key)
    trim = dict(trim_key)
    band_idx = {ji: n for n, ji in enumerate(band_list)}
    nb = max(1, len(band_list))
    nt = len(live_k)              # live k tiles
    Lk = nt * KT
    pos_of = {i: t for t, i in enumerate(live_k)}
    NSK = (Lk + 511) // 512       # kT column sub-blocks

    nc = bacc.Bacc("TRN2", target_bir_lowering=False, debug=False,
                   num_devices=N_CORES)

    # host-prelayouted inputs: [128, ...] sbuf-shaped flat rows
    qT = nc.dram_tensor("qT", [128, NKC * L], BF16, kind="ExternalInput")
    kvT = nc.dram_tensor("kvT", [128, NKC * Lk], BF16, kind="ExternalInput")
    wq = nc.dram_tensor("wq", [128, NKC * 512], BF16, kind="ExternalInput")
    # wkv: chunk k -> cols [256k:256k+128] = wk, [256k+128:256k+256] = wv
    wkv = nc.dram_tensor("wkv", [128, NKC * 256], BF16, kind="ExternalInput")
    # wo: gathered-row order (rank, pair, half, d) x own 512-col f-slab
    wo = nc.dram_tensor("wo", [128, NKC * 512], BF16, kind="ExternalInput")
    # consts_bf: cols 0:128 ones; row 0 cols 128:640 = bv4
    consts_bf = nc.dram_tensor("consts_bf", [128, 640], BF16,
                               kind="ExternalInput")
    # consts_f32: cols 0:4 bq, col 4 bk, cols 5:9 bo f-slab
    consts_f32 = nc.dram_tensor("consts_f32", [128, 21], F32,
                                kind="ExternalInput")
    band = nc.dram_tensor("band", [128, nb * LQC], BF16, kind="ExternalInput")

    ag_warm_in = nc.dram_tensor("ag_warm_in", [32, 16], BF16)
    ag_warm_out = nc.dram_tensor("ag_warm_out", [128, 16], BF16)
    ag_in = [nc.dram_tensor(f"ag_in{j}", [512, LQC], BF16)
             for j in range(NJ)]
    ag_out = [nc.dram_tensor(f"ag_out{j}", [4 * 512, LQC], BF16)
              for j in range(NJ)]
    out_ext = [nc.dram_tensor(f"out{j}", [512, LQC], BF16,
                              kind="ExternalOutput") for j in range(NJ)]
    groups = [[0, 1, 2, 3], [4, 5, 6, 7]]

    with tile.TileContext(nc) as tc:
        with tc.tile_pool(name="persist", bufs=1) as persist:
            # DMA plan (issue cost ~2.5us/DMA per queue -> few, big, ordered):
            #  sync:   consts_bf, kv pieces, consts_f32, wq, qt pieces, band,
            #          wo, final out copies
            #  scalar: wkv (everything else on scalar is exp)
            #  gpsimd: collective warmup + AG triggers + at exports + gathers
            cbf_sb = persist.tile([128, 640], BF16, tag="cbf")
            cf_sb = persist.tile([128, 21], F32, tag="cf")
            wq_sb = persist.tile([128, NKC * 512], BF16, tag="wq")
            wo_sb = persist.tile([128, NKC * 512], BF16, tag="wo")
            band_sb = persist.tile([128, nb * LQC], BF16, tag="band")
            kT_sb = persist.tile([128, Lk], BF16, tag="kT")
            v_sb = [persist.tile([128, 130], BF16, tag=f"v{t}", name=f"v{t}")
                    for t in range(nt)]
            QT_sb = [[persist.tile([128, 1024], BF16, tag=f"qt{m}{jp}",
                                   name=f"qt{m}{jp}") for jp in range(2)]
                     for m in range(NPAIR)]
            ones_sb = cbf_sb[:, 0:128]
            bv4_sb = cbf_sb[0:1, 128:640]
            bq_sb = cf_sb[:, 0:4]
            bk_sb = cf_sb[:, 4:5]
            bo_sb = cf_sb[:, 5:9]

            with tc.tile_pool(name="qtp_scope", bufs=1) as qtsc:
                wkv_sb = qtsc.tile([128, NKC * 256], BF16, tag="wkv")
                # qt staging read by the Q jp1 sub-eras inside attention
                # scope 1, so it lives at qtp_scope level.
                qtp = [[qtsc.tile([128, 8 * 1024], BF16, tag=f"qtp{jp}{h}",
                                  name=f"qtp{jp}{h}") for h in range(2)]
                       for jp in range(2)]

                nc.sync.dma_start(cbf_sb[:], consts_bf[:])
                nc.scalar.dma_start(wkv_sb[:], wkv[:])
                for t in range(nt):
                    nc.vector.tensor_copy(v_sb[t][:, 64:65], ones_sb[:, 0:1])
                    nc.vector.tensor_copy(v_sb[t][:, 129:130], ones_sb[:, 0:1])

                # ---- KV projection (kvch era-scoped; queue-ordered DMAs)
                with (
                    tc.tile_pool(name="kv_stream", bufs=1) as kvs,
                    tc.tile_pool(name="kv_psum", bufs=1, space="PSUM") as kvp,
                ):
                    # ALL critical input transfers go on the sync ring in
                    # exact consumption order -- SDMA round-robins *rings* at
                    # packet granularity, so priority only exists within one
                    # ring.
                    # first piece halved so the first matmul starts sooner
                    piece_of = [0, 0, 1, 1] + [2 + (k - 4) // 4
                                               for k in range(4, NKC)]
                    piece_off = [0, 2, 4, 8, 12]
                    piece_len = [2, 2, 4, 4, 4]
                    kvpc = [kvs.tile([128, piece_len[p] * Lk], BF16,
                                     tag=f"kvpc{p}", name=f"kvpc{p}")
                            for p in range(5)]
                    for p in range(5):
                        nc.sync.dma_start(
                            kvpc[p][:],
                            kvT[:, piece_off[p] * Lk:
                                (piece_off[p] + piece_len[p]) * Lk])
                    nc.sync.dma_start(cf_sb[:], consts_f32[:])
                    nc.sync.dma_start(wq_sb[:], wq[:])
                    for jp in range(2):
                        for h in range(2):
                            nc.sync.dma_start(
                                qtp[jp][h][:],
                                qT[:, (2 * jp + h) * 8192:
                                      (2 * jp + h + 1) * 8192])
                    # band/wo after the hot inputs on the same ring
                    nc.sync.dma_start(band_sb[:], band[:])
                    nc.sync.dma_start(wo_sb[:], wo[:])
                    nc.gpsimd.dma_start(ag_warm_in[:], consts_bf[0:32, 0:16])
                    nc.gpsimd.collective_compute(
                        "AllGather", mybir.AluOpType.bypass,
                        replica_groups=groups,
                        ins=[ag_warm_in[:]], outs=[ag_warm_out[:]])
                    wids = [min(512, Lk - 512 * s) for s in range(NSK)]
                    psk = [kvp.tile([128, wids[s]], F32, tag=f"psk{s}",
                                    name=f"psk{s}") for s in range(NSK)]
                    psv = [kvp.tile([128, wids[s]], F32, tag=f"psv{s}",
                                    name=f"psv{s}") for s in range(NSK)]
                    for s in range(NSK):
                        nc.tensor.matmul(psv[s][:], ones_sb[0:1, :],
                                         bv4_sb[:, 0:wids[s]], start=True,
                                         stop=False, skip_group_check=True)
                    for k in range(NKC):
                        kv_ch = kvpc[piece_of[k]]
                        off = (k - piece_off[piece_of[k]]) * Lk
                        for s in range(NSK):
                            nc.tensor.matmul(
                                psk[s][:], wkv_sb[:, 256 * k:256 * k + 128],
                                kv_ch[:, off + 512 * s:
                                      off + 512 * s + wids[s]],
                                start=(k == 0), stop=(k == NKC - 1))
                        for t in range(nt):
                            s, col = t // 4, t % 4
                            nc.tensor.matmul(
                                psv[s][:, 128 * col:128 * (col + 1)],
                                kv_ch[:, off + 128 * t:off + 128 * (t + 1)],
                                wkv_sb[:, 256 * k + 128:256 * (k + 1)],
                                start=False, stop=(k == NKC - 1),
                                skip_group_check=True)
                    for s in range(NSK):
                        nc.scalar.activation(
                            kT_sb[:, 512 * s:512 * s + wids[s]],
                            psk[s][:], AF.Identity, bias=bk_sb[:])
                    for t in range(nt):
                        s, col = t // 4, t % 4
                        nc.scalar.copy(v_sb[t][:, 0:64],
                                       psv[s][:, 128 * col:128 * col + 64])
                        nc.vector.tensor_copy(
                            v_sb[t][:, 65:129],
                            psv[s][:, 128 * col + 64:128 * (col + 1)])

                # ---- Q projection from resident qT (jp-major layout).
                # jp=1 is interleaved into attention chunk 0 (scope 1 below).
                with tc.tile_pool(name="q_psum", bufs=1, space="PSUM") as qp:
                    for jp in range(1):
                        psq = [qp.tile([128, 512], F32, tag=f"psq{n}",
                                       name=f"psq{n}") for n in range(8)]
                        for k in range(NKC):
                            qch = qtp[jp][k // 8]
                            off = (k % 8) * 1024
                            for m in range(NPAIR):
                                for jj in range(2):
                                    nc.tensor.matmul(
                                        psq[4 * jj + m][:],
                                        wq_sb[:, 512 * k + 128 * m:
                                                 512 * k + 128 * (m + 1)],
                                        qch[:, off + 512 * jj:
                                            off + 512 * (jj + 1)],
                                        start=(k == 0), stop=(k == NKC - 1))
                        for jj in range(2):
                            for m in range(NPAIR):
                                nc.scalar.activation(
                                    QT_sb[m][jp][:, 512 * jj:512 * (jj + 1)],
                                    psq[4 * jj + m][:], AF.Identity,
                                    bias=bq_sb[:, m:m + 1])

                # ---- Attention scope 1: chunk 0 with Q jp=1 sub-eras.
                P = {}
                # two-stage deferred normalize for scope 2: stage A broadcasts
                # the reciprocal row via a PE ones-matmul into PSUM one block
                # after its recips; stage B (muls + at->DRAM exports) runs one
                # block after that, so no PE-feeding queue ever waits on a
                # laggy cross-engine producer.
                norm_bc = []   # (ra, rb, ua, ub, dst, row0)
                norm_mul = []  # (ua, ub, rba, rbb, dst, row0)

                def flush_norm():
                    if norm_mul:
                        ua, ub, rba, rbb, dst, row0 = norm_mul.pop(0)
                        at_a = P["at_pool"].tile([64, 512], BF16, tag="at_a")
                        at_b = P["at_pool"].tile([64, 512], BF16, tag="at_b")
                        nc.vector.tensor_mul(at_a[:], ua[0:64, :], rba[:])
                        nc.vector.tensor_mul(at_b[:], ub[0:64, :], rbb[:])
                        nc.gpsimd.dma_start(dst[row0:row0 + 64, :], at_a[:])
                        nc.gpsimd.dma_start(dst[row0 + 64:row0 + 128, :],
                                            at_b[:])
                    if norm_bc:
                        ra, rb, ua, ub, dst, row0 = norm_bc.pop(0)
                        rba = P["bc_psum"].tile([64, 512], F32, tag="rba")
                        rbb = P["bc_psum"].tile([64, 512], F32, tag="rbb")
                        nc.tensor.matmul(rba[:], ones_sb[0:1, 0:64], ra[:],
                                         start=True, stop=True,
                                         skip_group_check=True)
                        nc.tensor.matmul(rbb[:], ones_sb[0:1, 0:64], rb[:],
                                         start=True, stop=True,
                                         skip_group_check=True)
                        norm_mul.append((ua, ub, rba, rbb, dst, row0))

                def drain_norm():
                    while norm_mul or norm_bc:
                        flush_norm()

                def attn_block(j, pr, dst, dst_row0):
                    """Compute attention for (chunk j, pair pr); stage the two
                    normalized 64-row halves for export to DRAM tensor dst at
                    rows dst_row0 / dst_row0+64 (finished by the next
                    flush_norm)."""
                    lv = live[j]
                    nlast = len(lv) - 1
                    pva = P["pv_psum"].tile([65, 512], F32, tag="pva")
                    pvb = P["pv_psum"].tile([65, 512], F32, tag="pvb")
                    pts = []
                    for n, i in enumerate(lv):
                        t = pos_of[i]
                        q0 = trim.get((j, i), 0)
                        qt_t = QT_sb[pr][j // 2]
                        qoff = 512 * (j % 2)
                        ps = P["qk_psum"].tile([128, 1024], F32, tag="qk")
                        nc.tensor.matmul(
                            ps[:, q0:512],
                            kT_sb[0:64, 128 * t:128 * (t + 1)],
                            qt_t[0:64, qoff + q0:qoff + 512],
                            start=True, stop=True, skip_group_check=True)
                        nc.tensor.matmul(
                            ps[:, 512 + q0:1024],
                            kT_sb[64:128, 128 * t:128 * (t + 1)],
                            qt_t[64:128, qoff + q0:qoff + 512],
                            start=True, stop=True, skip_group_check=True)
                        pt = P["pt_pool"].tile([128, 1024], BF16, tag="pt")
                        if q0 == 0:
                            nc.scalar.activation(pt[:], ps[:], AF.Exp)
                        else:
                            nc.scalar.activation(pt[:, q0:512],
                                                 ps[:, q0:512], AF.Exp)
                            nc.scalar.activation(pt[:, 512 + q0:1024],
                                                 ps[:, 512 + q0:1024], AF.Exp)
                        if (j, i) in band_idx:
                            bcol = band_idx[(j, i)] * LQC
                            nc.vector.tensor_mul(
                                pt[:, q0:512], pt[:, q0:512],
                                band_sb[:, bcol + q0:bcol + 512])
                            nc.vector.tensor_mul(
                                pt[:, 512 + q0:1024], pt[:, 512 + q0:1024],
                                band_sb[:, bcol + q0:bcol + 512])
                        pts.append((pt, q0))
                        if n == 2:
                            # previous block's normalize finish lands here,
                            # giving its broadcast a full block of slack
                            flush_norm()

                        def pv_step(m):
                            ptp, q0p = pts[m]
                            tp = pos_of[lv[m]]
                            nc.tensor.matmul(
                                pva[:, q0p:512], v_sb[tp][:, 0:65],
                                ptp[:, q0p:512], start=(m == 0),
                                stop=(m == nlast), skip_group_check=True)
                            nc.tensor.matmul(
                                pvb[:, q0p:512], v_sb[tp][:, 65:130],
                                ptp[:, 512 + q0p:1024], start=(m == 0),
                                stop=(m == nlast), skip_group_check=True)

                        # software pipeline: PV lags QK/exp by two steps so
                        # the PE never catches up with exp+band (pt bufs=3)
                        if n > 1:
                            pv_step(n - 2)
                    pv_step(nlast - 1)
                    pv_step(nlast)
                    # unload + normalize (vector only; scalar stays pure exp)
                    ua = P["ua_pool"].tile([65, 512], F32, tag="ua")
                    ub = P["ua_pool"].tile([65, 512], F32, tag="ub")
                    nc.vector.tensor_copy(ua[:], pva[:])
                    nc.vector.tensor_copy(ub[:], pvb[:])
                    # den rows to partition 0 (DVE cannot partition-shift
                    # SBUF->SBUF; PSUM row reads to p0 are fine)
                    rsa = P["rc_pool"].tile([1, 512], F32, tag="rsa")
                    rsb = P["rc_pool"].tile([1, 512], F32, tag="rsb")
                    nc.vector.tensor_copy(rsa[:], pva[64:65, :])
                    nc.vector.tensor_copy(rsb[:], pvb[64:65, :])
                    ra = P["rc_pool"].tile([1, 512], F32, tag="ra")
                    rb = P["rc_pool"].tile([1, 512], F32, tag="rb")
                    nc.vector.reciprocal_approx_fast(out=ra[:], in_=rsa[:])
                    nc.vector.reciprocal_approx_fast(out=rb[:], in_=rsb[:])
                    if P.get("defer"):
                        # bf16 copies feed the PE ones-matmul broadcast
                        rah = P["rc_pool"].tile([1, 512], BF16, tag="rah")
                        rbh = P["rc_pool"].tile([1, 512], BF16, tag="rbh")
                        nc.vector.tensor_copy(rah[:], ra[:])
                        nc.vector.tensor_copy(rbh[:], rb[:])
                        norm_bc.append((rah, rbh, ua, ub, dst, dst_row0))
                    else:
                        # chunk 0: no collective in flight, gpsimd broadcast
                        # + immediate finish is safe and cheapest
                        rba = P["rb_pool"].tile([64, 512], F32, tag="rba")
                        rbb = P["rb_pool"].tile([64, 512], F32, tag="rbb")
                        nc.gpsimd.partition_broadcast(rba[:], ra[:])
                        nc.gpsimd.partition_broadcast(rbb[:], rb[:])
                        at_a = P["at_pool"].tile([64, 512], BF16, tag="at_a")
                        at_b = P["at_pool"].tile([64, 512], BF16, tag="at_b")
                        nc.vector.tensor_mul(at_a[:], ua[0:64, :], rba[:])
                        nc.vector.tensor_mul(at_b[:], ub[0:64, :], rbb[:])
                        nc.gpsimd.dma_start(
                            dst[dst_row0:dst_row0 + 64, :], at_a[:])
                        nc.gpsimd.dma_start(
                            dst[dst_row0 + 64:dst_row0 + 128, :], at_b[:])

                def q_sub_era(s, qsub):
                    jj, mp = s // 2, s % 2
                    psq = qsub.tile([128, 1024], F32, tag="qsub")
                    for k in range(NKC):
                        qch = qtp[1][k // 8]
                        off = (k % 8) * 1024 + 512 * jj
                        for mi in range(2):
                            m = 2 * mp + mi
                            nc.tensor.matmul(
                                psq[:, 512 * mi:512 * (mi + 1)],
                                wq_sb[:, 512 * k + 128 * m:
                                         512 * k + 128 * (m + 1)],
                                qch[:, off:off + 512],
                                start=(k == 0), stop=(k == NKC - 1))
                    for mi in range(2):
                        m = 2 * mp + mi
                        nc.scalar.activation(
                            QT_sb[m][1][:, 512 * jj:512 * (jj + 1)],
                            psq[:, 512 * mi:512 * (mi + 1)], AF.Identity,
                            bias=bq_sb[:, m:m + 1])

                with (
                    tc.tile_pool(name="pt1", bufs=3) as _pt1,
                    tc.tile_pool(name="ua1", bufs=2) as _ua1,
                    tc.tile_pool(name="rc1", bufs=2) as _rc1,
                    tc.tile_pool(name="rb1", bufs=2) as _rb1,
                    tc.tile_pool(name="at1", bufs=4) as _at1,
                    tc.tile_pool(name="qk1", bufs=2, space="PSUM") as _qk1,
                    tc.tile_pool(name="pv1", bufs=1, space="PSUM") as _pv1,
                    tc.tile_pool(name="qsub", bufs=1, space="PSUM") as _qs,
                ):
                    P.update(pt_pool=_pt1, ua_pool=_ua1, rc_pool=_rc1,
                             rb_pool=_rb1, at_pool=_at1,
                             qk_psum=_qk1, pv_psum=_pv1, defer=False)
                    for pr in range(NPAIR):
                        attn_block(0, pr, ag_in[0], 128 * pr)
                        q_sub_era(pr, _qs)
                    nc.gpsimd.collective_compute(
                        "AllGather", mybir.AluOpType.bypass,
                        replica_groups=groups,
                        ins=[ag_in[0][:]], outs=[ag_out[0][:]])

            # ---- Scope 2: chunks 1..3 + AGs, then deferred oproj tail.
            # atf holds all four gathered chunks (read only in the tail).
            with (
                tc.tile_pool(name="atf", bufs=3) as _atf,
                tc.tile_pool(name="atf3", bufs=1) as _atf3,
            ):
                def gather_to_sbuf(pool, src_dram, a, tag):
                    # gpsimd: queued right behind the AllGather that produces
                    # src_dram, so it issues the moment the AG completes
                    t = pool.tile([128, a * 512], BF16, tag=tag)
                    dst_ap = t[:].rearrange("p (a c) -> p a c", a=a)
                    src_ap = src_dram[:].rearrange("(a p) c -> p a c", p=128)
                    nc.gpsimd.dma_start(dst_ap, src_ap)
                    return t

                atf = [None] * NJ

                with (
                    tc.tile_pool(name="pt2", bufs=3) as _pt2,
                    tc.tile_pool(name="ua2", bufs=3) as _ua2,
                    tc.tile_pool(name="rc2", bufs=2) as _rc2,
                    tc.tile_pool(name="at2", bufs=6) as _at2,
                    tc.tile_pool(name="qk2", bufs=2, space="PSUM") as _qk2,
                    tc.tile_pool(name="pv2", bufs=1, space="PSUM") as _pv2,
                    tc.tile_pool(name="bc2", bufs=1, space="PSUM") as _bc2,
                ):
                    P.update(pt_pool=_pt2, ua_pool=_ua2, rc_pool=_rc2,
                             at_pool=_at2, bc_psum=_bc2,
                             qk_psum=_qk2, pv_psum=_pv2, defer=True)
                    atf[0] = gather_to_sbuf(_atf, ag_out[0], 16, "atf")

                    atf3b = [None]
                    for j in range(1, NJ):
                        for pr in range(NPAIR):
                            attn_block(j, pr, ag_in[j], 128 * pr)
                        drain_norm()
                        nc.gpsimd.collective_compute(
                            "AllGather", mybir.AluOpType.bypass,
                            replica_groups=groups,
                            ins=[ag_in[j][:]], outs=[ag_out[j][:]])
                        if j < NJ - 1:
                            atf[j] = gather_to_sbuf(_atf, ag_out[j], 16,
                                                    "atf")
                        else:
                            # split the last gather so oproj(3)'s first half
                            # of the contraction starts as soon as possible
                            atf[j] = gather_to_sbuf(
                                _atf3, ag_out[j][0:1024, :], 8, "atf3a")
                            atf3b[0] = gather_to_sbuf(
                                _atf3, ag_out[j][1024:2048, :], 8, "atf3b")

                # ---- Deferred o-projection tail: local f-slab per chunk
                # (own PSUM scope; the attention pools above are closed).
                with (
                    tc.tile_pool(name="osb2", bufs=2) as _osb2,
                    tc.tile_pool(name="o_ps", bufs=2, space="PSUM") as _ops,
                ):
                    for j in range(NJ):
                        osb = _osb2.tile([128, 4 * 512], BF16, tag="osb")
                        for fb in range(4):
                            pso = _ops.tile([128, 512], F32, tag="pso")
                            for dc in range(16):
                                if j == NJ - 1 and dc >= 8:
                                    mv = atf3b[0][:, 512 * (dc - 8):
                                                  512 * (dc - 7)]
                                else:
                                    mv = atf[j][:, 512 * dc:512 * (dc + 1)]
                                wcol = 512 * dc + 128 * fb
                                nc.tensor.matmul(
                                    pso[:], wo_sb[:, wcol:wcol + 128], mv,
                                    start=(dc == 0), stop=(dc == 15),
                                    skip_group_check=True)
                            sl = osb[:, 512 * fb:512 * (fb + 1)]
                            if fb % 2 == 0:
                                nc.scalar.activation(sl, pso[:], AF.Identity,
                                                     bias=bo_sb[:, fb:fb + 1])
                            else:
                                nc.vector.tensor_scalar_add(
                                    sl, pso[:], bo_sb[:, fb:fb + 1])
                        dst_ap = out_ext[j][:].rearrange("(a p) c -> p a c",
                                                         p=128)
                        src_ap = osb[:].rearrange("p (a c) -> p a c", a=4)
                        nc.sync.dma_start(dst_ap, src_ap)

    nc.compile()
    _graph_cache[key] = nc
    return nc


def _prelayout(a, width):
    """[NKC*128, width] row-major -> [128, NKC*width] sbuf layout."""
    return np.ascontiguousarray(
        a.reshape(NKC, 128, width).transpose(1, 0, 2).reshape(128, NKC * width))


def kernel(query, kv, Wq, bq, Wkv, bkv, Wo, bo, attn_mask, key_padding_mask):
    global last_results
    query = np.asarray(query, np.float32)
    kv = np.asarray(kv, np.float32)
    Wq = np.asarray(Wq, np.float32)
    bq = np.asarray(bq, np.float32)
    Wkv = np.asarray(Wkv, np.float32)
    bkv = np.asarray(bkv, np.float32)
    Wo = np.asarray(Wo, np.float32)
    bo = np.asarray(bo, np.float32)
    attn_mask = np.asarray(attn_mask, np.float32)
    kpm = np.asarray(key_padding_mask)

    eff = [attn_mask + np.where(kpm[b], np.float32(-1e9),
                                np.float32(0.0))[None, :]
           for b in range(B)]
    live, band_list, trim = _classify_blocks(eff)
    live_k = sorted({i for lv in live.values() for i in lv})
    live_key = tuple((j, tuple(lv)) for j, lv in sorted(live.items()))
    band_key = tuple(band_list)
    trim_key = tuple(sorted(trim.items()))

    nc = _build_graph(live_key, band_key, trim_key, live_k)

    nt = len(live_k)
    Lk = nt * KT

    # Host-side shard prep. qT jp-major: col = jp*16*1024 + k*1024 + c
    qTh = [np.ascontiguousarray(
        query[b].T.astype(NPBF16).reshape(NKC, 128, 2, 1024)
        .transpose(2, 1, 0, 3).reshape(2, 128, NKC * 1024)
        .transpose(1, 0, 2).reshape(128, NKC * L)) for b in range(B)]
    kvTsel = [np.ascontiguousarray(
        kv[b].T.astype(NPBF16)
        .reshape(HID, NI, KT)[:, live_k, :].reshape(HID, Lk)) for b in range(B)]
    kvTh = [_prelayout(k_, Lk) for k_ in kvTsel]
    nb = max(1, len(band_list))
    bandh = []
    with np.errstate(over="ignore", under="ignore"):
        for b in range(B):
            if band_list:
                bandh.append(np.ascontiguousarray(np.concatenate(
                    [np.exp(eff[b][j * LQC:(j + 1) * LQC,
                                   i * KT:(i + 1) * KT].T)
                     for (j, i) in band_list], axis=1).astype(NPBF16)))
            else:
                bandh.append(np.zeros((KT, nb * LQC), NPBF16))
    ones_h = np.ones((128, 128), NPBF16)

    Wq_h = Wq.reshape(HID, NH, D)
    bq_h = bq.reshape(NH, D)

    # gathered-row permutation: global row g = 512*rank + 128*pair + 64*e + d
    g = np.arange(HID)
    head_of_g = 8 * (g // 512) + (g % 512) // 128 + 4 * ((g % 128) // 64)
    row_of_g = head_of_g * D + (g % 64)

    in_maps = []
    for c in range(N_CORES):
        b, r = c // TPR, c % TPR
        heads_q = [8 * r + pr + 4 * e for pr in range(NPAIR) for e in range(2)]
        wq_c = _prelayout(
            (Wq_h[:, heads_q, :].reshape(HID, 512) * SCALE).astype(NPBF16),
            512)
        bq_c = (bq_h[heads_q].reshape(512) * SCALE).reshape(4, 128).T
        wk_c = Wkv[:, 128 * r:128 * (r + 1)].astype(NPBF16)
        bk_c = bkv[128 * r:128 * (r + 1)]
        wv_c = Wkv[:, 512 + 128 * r:512 + 128 * (r + 1)].astype(NPBF16)
        # wkv interleave: chunk k -> [wk_k | wv_k]
        wkv_c = np.ascontiguousarray(np.concatenate(
            [np.concatenate([wk_c.reshape(NKC, 128, 128)[k],
                             wv_c.reshape(NKC, 128, 128)[k]], axis=1)
             for k in range(NKC)], axis=1))  # [128, NKC*256]
        bv_c = bkv[512 + 128 * r:512 + 128 * (r + 1)]
        # wo: gathered-row order x own f-slab columns
        wo_c = _prelayout(
            Wo[row_of_g, 512 * r:512 * (r + 1)].astype(NPBF16), 512)
        cbf = np.zeros((128, 640), NPBF16)
        cbf[:, 0:128] = ones_h
        cbf[0, 128:640] = np.tile(bv_c, 4).astype(NPBF16)
        cf = np.zeros((128, 21), np.float32)
        cf[:, 0:4] = bq_c
        cf[:, 4] = bk_c
        cf[:, 5:9] = bo[512 * r:512 * (r + 1)].reshape(4, 128).T
        in_maps.append({
            "qT": qTh[b], "kvT": kvTh[b],
            "wq": wq_c, "wkv": wkv_c, "wo": wo_c,
            "consts_bf": np.ascontiguousarray(cbf),
            "consts_f32": np.ascontiguousarray(cf),
            "band": bandh[b],
        })

    last_results = run_bass_kernel_spmd(nc, in_maps,
                                        core_ids=list(range(N_CORES)))

    out = np.empty((B, L, HID), np.float32)
    for c in range(N_CORES):
        b, r = c // TPR, c % TPR
        for j in range(NJ):
            out[b, 512 * j:512 * (j + 1), 512 * r:512 * (r + 1)] = \
                last_results.results[c][f"out{j}"].T.astype(np.float32)
    return out
